# revision 1
# baseline (speedup 1.0000x reference)
"""Trainium2 Bass kernel for paged causal self-attention (GQA + YaRN rope).

Sharding: tensor-parallel over heads. Core c (of 8) owns kv-head c and
q-heads 2c, 2c+1 for both batches. Each core computes a partial output
y_c = attn_c @ Wo_c.T over its 256 channels; the host sums the 8 partials.

The reference's scatter of new K/V into the pools is dead code w.r.t. the
returned output (slot_map is a permutation, so gathered past slots are
disjoint from the scattered new slots); new K/V are consumed directly from
SBUF and only the past 1024 slots per batch are gathered via indirect DMA.

Matmuls run as float32r (full PE rate at free dim >= 256). The BIR verifier
requires every fp32r matmul operand to be produced by a compute op that
rounds to fp32r, so DMA-loaded operands pass through a rounding copy and
intermediate tiles are declared float32r at their producing op.
"""

import sys

sys.path.insert(0, "/opt/trn_rl_repo")

import numpy as np

import concourse.bacc as bacc
import concourse.bass as bass
import concourse.tile as tile
from concourse import mybir
from concourse.bass import IndirectOffsetOnAxis
from concourse.bass_utils import run_bass_kernel_spmd

F32 = mybir.dt.float32
F32R = mybir.dt.float32r
I32 = mybir.dt.int32
EXP = mybir.ActivationFunctionType.Exp

B, T, PAST = 2, 1024, 1024
H, HKV, D = 16, 8, 128
G = H // HKV            # q heads per kv head
C = H * D               # 2048
TOTAL = PAST + T        # 2048
NB = B * T              # 2048 flattened tokens
NCORES = 8
P = 128
TB = 512                # token block for projections
NEG = -1.0e30


def _f(ap):
    return ap.bitcast(F32)


def _emit(tc, io):
    nc = tc.nc
    (xT, wq, wk, wv, wo, kp, vp, gidx, cosq, sinq, cosk, sink,
     cmask, rperm, ident, ones, y) = io

    with (
        tc.tile_pool(name="const", bufs=1) as cp,
        tc.tile_pool(name="persist", bufs=1) as pp,
        tc.tile_pool(name="gather", bufs=1) as gp,
        tc.tile_pool(name="ysb", bufs=3) as yp,
    ):
        # ---- constants ----
        gidx_t = cp.tile([P, 2 * 8], I32)
        nc.sync.dma_start(gidx_t[:], gidx[:])
        cmask_t = cp.tile([P, 4, TB], F32)
        nc.sync.dma_start(cmask_t[:], cmask[:])
        ident_t = cp.tile([P, P], F32)
        nc.sync.dma_start(ident_t[:], ident[:])
        rperm_s = cp.tile([P, P], F32)
        nc.sync.dma_start(rperm_s[:], rperm[:])
        rperm_t = cp.tile([P, P], F32R)
        nc.vector.tensor_copy(rperm_t[:], rperm_s[:])
        ones_s = cp.tile([P, P], F32)
        nc.sync.dma_start(ones_s[:], ones[:])
        ones_t = cp.tile([P, P], F32R)
        nc.vector.tensor_copy(ones_t[:], ones_s[:])

        vgr = [None, None]

        # ---- persistent activations (float32r: producers are compute ops) ----
        qT0 = pp.tile([P, NB], F32R)      # q head 2c,   [d, token]
        qT1 = pp.tile([P, NB], F32R)      # q head 2c+1
        kT_new = pp.tile([P, NB], F32R)   # new keys,    [d, token]
        v_nat = pp.tile([P, B, 8, P], F32R)    # new values, [t%128, b, chunk, d]
        kT_past = pp.tile([P, B, 8, P], F32R)  # past keys,  [d, b, chunk, s%128]
        wo_t = pp.tile([P, G, C], F32R)   # rounded Wo slice

        # ================= phase 1: projections + rope =================
        with (
            tc.tile_pool(name="tabs", bufs=1) as tbp,
            tc.tile_pool(name="kgvg", bufs=1) as kvp,
            tc.tile_pool(name="wts", bufs=1) as wp,
            tc.tile_pool(name="xt", bufs=3) as xp,
            tc.tile_pool(name="rope", bufs=2) as rp,
            tc.tile_pool(name="pproj", bufs=1, space="PSUM") as pjp,
            tc.tile_pool(name="prope", bufs=2, space="PSUM") as rpp,
            tc.tile_pool(name="ptr", bufs=2, space="PSUM") as trp,
        ):
            # past K/V gather (emit early; DMA overlaps weight loads)
            kg = [None, None]
            for b in range(B):
                kg[b] = kvp.tile([P, 8, P], F32, name=f"kg{b}", tag=f"kg{b}")
                vg = kvp.tile([P, 8, P], F32, name=f"vg{b}", tag=f"vg{b}")
                for j in range(8):
                    # [P,1]-index gathers: the multi-column offset-AP form
                    # miscomputes on hardware
                    nc.gpsimd.indirect_dma_start(
                        out=kg[b][:, j, :],
                        out_offset=None,
                        in_=kp[:, :],
                        in_offset=IndirectOffsetOnAxis(
                            ap=gidx_t[:, 8 * b + j:8 * b + j + 1], axis=0),
                    )
                    nc.gpsimd.indirect_dma_start(
                        out=vg[:, j, :],
                        out_offset=None,
                        in_=vp[:, :],
                        in_offset=IndirectOffsetOnAxis(
                            ap=gidx_t[:, 8 * b + j:8 * b + j + 1], axis=0),
                    )
                vgr[b] = gp.tile([P, 8, P], F32R, name=f"vgr{b}", tag=f"vgr{b}")
                nc.vector.tensor_copy(vgr[b][:], vg[:])

            # rope tables (phase-1 only)
            cosq_t = tbp.tile([P, T], F32)
            nc.sync.dma_start(cosq_t[:], cosq[:])
            sinq_t = tbp.tile([P, T], F32)
            nc.sync.dma_start(sinq_t[:], sinq[:])
            cosk_t = tbp.tile([P, T], F32)
            nc.sync.dma_start(cosk_t[:], cosk[:])
            sink_t = tbp.tile([P, T], F32)
            nc.sync.dma_start(sink_t[:], sink[:])

            # weights: stage in [P,4,P] pieces through the xts slot rotation
            # (24 pieces = 3 full cycles of 8 slots, keeping the HWDGE
            # round-robin aligned so slot-reuse WAW deps stay same-proc)
            wq_t = wp.tile([P, 16, 2 * P], F32R)
            wk_t = wp.tile([P, 16, P], F32R)
            wv_t = wp.tile([P, 16, P], F32R)
            wqr = wq.rearrange("(kc p) m -> p kc m", p=P)
            wkr = wk.rearrange("(kc p) m -> p kc m", p=P)
            wvr = wv.rearrange("(kc p) m -> p kc m", p=P)
            wor = wo.rearrange("(g p) (q m) -> p g q m", p=P, m=P)  # [P,2,16,128]
            pieces = []
            for hh in range(2):
                for q4 in range(4):
                    pieces.append((wqr[:, 4 * q4:4 * q4 + 4, hh * P:(hh + 1) * P],
                                   wq_t[:, 4 * q4:4 * q4 + 4, hh * P:(hh + 1) * P]))
            for q4 in range(4):
                pieces.append((wkr[:, 4 * q4:4 * q4 + 4, :],
                               wk_t[:, 4 * q4:4 * q4 + 4, :]))
            for q4 in range(4):
                pieces.append((wvr[:, 4 * q4:4 * q4 + 4, :],
                               wv_t[:, 4 * q4:4 * q4 + 4, :]))
            wot4 = wo_t.rearrange("p g (q m) -> p g q m", m=P)   # [P,2,16,128]
            for g in range(G):
                for q4 in range(4):
                    pieces.append((wor[:, g, 4 * q4:4 * q4 + 4, :],
                                   wot4[:, g, 4 * q4:4 * q4 + 4, :]))
            for src_ap, dst_ap in pieces:
                w_s = xp.tile([P, 4, P], F32, name="w_s", tag="xts", bufs=8)
                nc.sync.dma_start(w_s[:], src_ap)
                nc.vector.tensor_copy(dst_ap, w_s[:])

            for tb in range(NB // TB):           # 4 token blocks of 512
                n0 = tb * TB
                b = tb // 2
                tpos = (tb % 2) * TB             # position-in-batch of block start

                q0p = pjp.tile([P, TB], F32, name="q0p", tag="q0")
                q1p = pjp.tile([P, TB], F32, name="q1p", tag="q1")
                kkp = pjp.tile([P, TB], F32, name="kkp", tag="kk")
                vvp = pjp.tile([P, TB], F32, name="vvp", tag="vv")
                for kc in range(16):
                    xt_s = xp.tile([P, TB], F32, name="xt_s", tag="xts", bufs=8)
                    nc.sync.dma_start(xt_s[:], xT[kc * P:(kc + 1) * P, n0:n0 + TB])
                    xt = xp.tile([P, TB], F32R, name="xt", tag="xt")
                    nc.vector.tensor_copy(xt[:], xt_s[:])
                    st = (kc == 0)
                    sp = (kc == 15)
                    nc.tensor.matmul(q0p[:], wq_t[:, kc, 0:P], xt[:], start=st, stop=sp)
                    nc.tensor.matmul(q1p[:], wq_t[:, kc, P:2 * P], xt[:], start=st, stop=sp)
                    nc.tensor.matmul(kkp[:], wk_t[:, kc, :], xt[:], start=st, stop=sp)
                    nc.tensor.matmul(vvp[:], wv_t[:, kc, :], xt[:], start=st, stop=sp)

                # rope for q0, q1, k
                for src, dst, ct, stt in (
                    (q0p, qT0, cosq_t, sinq_t),
                    (q1p, qT1, cosq_t, sinq_t),
                    (kkp, kT_new, cosk_t, sink_t),
                ):
                    raw = rp.tile([P, TB], F32R, name="raw", tag="raw")
                    nc.scalar.copy(raw[:], src[:])
                    rot = rpp.tile([P, TB], F32, name="rot", tag="rot")
                    nc.tensor.matmul(rot[:], rperm_t[:], raw[:], start=True, stop=True)
                    dslice = dst[:, n0:n0 + TB]
                    nc.vector.tensor_mul(dslice, _f(raw[:]), ct[:, tpos:tpos + TB])
                    tmp = rp.tile([P, TB], F32, name="tmp", tag="tmp")
                    nc.vector.tensor_mul(tmp[:], rot[:], stt[:, tpos:tpos + TB])
                    nc.vector.tensor_add(dslice, _f(dslice), tmp[:])

                # v: no rope; transpose [d, t] -> [t, d] in 128-chunks
                vraw = rp.tile([P, TB], F32, name="vraw", tag="vraw")
                nc.scalar.copy(vraw[:], vvp[:])
                for j4 in range(TB // P):
                    vt = trp.tile([P, P], F32, name="vt", tag="tr")
                    nc.tensor.transpose(vt[:], vraw[:, j4 * P:(j4 + 1) * P],
                                        ident_t[:])
                    nc.vector.tensor_copy(v_nat[:, b, (tb % 2) * 4 + j4, :], vt[:])

            # past K transpose: [s, d] -> [d, s]
            for b in range(B):
                for j in range(8):
                    kt = trp.tile([P, P], F32, name="kt", tag="tr")
                    nc.tensor.transpose(kt[:], kg[b][:, j, :], ident_t[:])
                    nc.vector.tensor_copy(kT_past[:, b, j, :], kt[:])

        # ================= phase 3+4: attention + output proj =================
        with (
            tc.tile_pool(name="attp", bufs=1) as ap_,
            tc.tile_pool(name="exps", bufs=1) as ep,
            tc.tile_pool(name="sums", bufs=2) as sp_,
            tc.tile_pool(name="pscore", bufs=2, space="PSUM") as scp,
            tc.tile_pool(name="pav", bufs=2, space="PSUM") as avp,
            tc.tile_pool(name="psum1", bufs=1, space="PSUM") as s1p,
            tc.tile_pool(name="pbc", bufs=1, space="PSUM") as bcp,
            tc.tile_pool(name="py", bufs=2, space="PSUM") as pyp,
        ):
            att0 = ap_.tile([P, NB], F32R)    # attention out head 2c, [d, token]
            att1 = ap_.tile([P, NB], F32R)

            for b in range(B):
                for tbq in range(2):             # query block of 512 within batch
                    t0 = b * T + tbq * TB        # global token offset
                    for g, (qT, att) in enumerate(((qT0, att0), (qT1, att1))):
                        q_ap = qT[:, t0:t0 + TB]
                        njnew = 4 * tbq + 4
                        nch = 8 + njnew
                        expS = ep.tile([P, 16, TB], F32R, name="expS", tag="expS")
                        sumP = sp_.tile([P, TB], F32R, name="sumP", tag="sumP")
                        av = avp.tile([P, TB], F32, name="av", tag="av")

                        chunks = [(kT_past[:, b, j, :], vgr[b][:, j, :], None)
                                  for j in range(8)]
                        for j in range(njnew):
                            koff = b * T + j * P
                            ri = j - 4 * tbq
                            chunks.append((kT_new[:, koff:koff + P],
                                           v_nat[:, b, j, :],
                                           ri if ri >= 0 else None))

                        for ci, (k_ap, v_ap, mri) in enumerate(chunks):
                            s_ps = scp.tile([P, TB], F32, name="s_ps", tag="s")
                            nc.tensor.matmul(s_ps[:], k_ap, q_ap,
                                             start=True, stop=True)
                            if mri is not None:
                                nc.vector.tensor_add(s_ps[:], s_ps[:],
                                                     cmask_t[:, mri, :])
                            e_ap = expS[:, ci, :]
                            nc.scalar.activation(e_ap, s_ps[:], EXP)
                            if ci == 0:
                                nc.vector.tensor_copy(sumP[:], _f(e_ap))
                            else:
                                nc.vector.tensor_add(sumP[:], _f(sumP[:]), _f(e_ap))
                            nc.tensor.matmul(av[:], v_ap, e_ap,
                                             start=(ci == 0), stop=(ci == nch - 1))

                        # softmax denominator: reduce over partitions + bcast
                        tsum = s1p.tile([1, TB], F32, name="tsum", tag="t1")
                        nc.tensor.matmul(tsum[:], ones_t[:, 0:1], sumP[:],
                                         start=True, stop=True)
                        ssb = sp_.tile([1, TB], F32, name="ssb", tag="ssb")
                        nc.scalar.copy(ssb[:], tsum[:])
                        rinv = sp_.tile([1, TB], F32, name="rinv", tag="rinv")
                        nc.vector.reciprocal(rinv[:], ssb[:])
                        rinvr = sp_.tile([1, TB], F32R, name="rinvr", tag="rinvr")
                        nc.vector.tensor_copy(rinvr[:], rinv[:])
                        rbc = bcp.tile([P, TB], F32, name="rbc", tag="rbc")
                        nc.tensor.matmul(rbc[:], ones_t[0:1, :], rinvr[:],
                                         start=True, stop=True)
                        rbs = sp_.tile([P, TB], F32, name="rbs", tag="rbs")
                        nc.scalar.copy(rbs[:], rbc[:])
                        nc.vector.tensor_mul(att[:, t0:t0 + TB], av[:], rbs[:])

                    # output projection for these 512 tokens (4 chunks of 128)
                    for tc4 in range(4):
                        tt0 = t0 + tc4 * P
                        for cb in range(4):
                            yps = pyp.tile([P, TB], F32, name="yps", tag="y")
                            nc.tensor.matmul(yps[:], att0[:, tt0:tt0 + P],
                                             wo_t[:, 0, cb * TB:(cb + 1) * TB],
                                             start=True, stop=False)
                            nc.tensor.matmul(yps[:], att1[:, tt0:tt0 + P],
                                             wo_t[:, 1, cb * TB:(cb + 1) * TB],
                                             start=False, stop=True)
                            ysb = yp.tile([P, TB], F32, name="ysbt", tag="ysbt")
                            nc.scalar.copy(ysb[:], yps[:])
                            nc.sync.dma_start(
                                y[tt0:tt0 + P, cb * TB:(cb + 1) * TB], ysb[:])


def build_nc():
    nc = bacc.Bacc("TRN2")
    xT = nc.dram_tensor("xT", [C, NB], F32, kind="ExternalInput")
    wq = nc.dram_tensor("wq", [C, G * D], F32, kind="ExternalInput")
    wk = nc.dram_tensor("wk", [C, D], F32, kind="ExternalInput")
    wv = nc.dram_tensor("wv", [C, D], F32, kind="ExternalInput")
    wo = nc.dram_tensor("wo", [G * D, C], F32, kind="ExternalInput")
    kp = nc.dram_tensor("kp", [B * TOTAL, D], F32, kind="ExternalInput")
    vp = nc.dram_tensor("vp", [B * TOTAL, D], F32, kind="ExternalInput")
    gidx = nc.dram_tensor("gidx", [P, B * 8], I32, kind="ExternalInput")
    cosq = nc.dram_tensor("cosq", [P, T], F32, kind="ExternalInput")
    sinq = nc.dram_tensor("sinq", [P, T], F32, kind="ExternalInput")
    cosk = nc.dram_tensor("cosk", [P, T], F32, kind="ExternalInput")
    sink = nc.dram_tensor("sink", [P, T], F32, kind="ExternalInput")
    cmask = nc.dram_tensor("cmask", [P, 4, TB], F32, kind="ExternalInput")
    rperm = nc.dram_tensor("rperm", [P, P], F32, kind="ExternalInput")
    ident = nc.dram_tensor("ident", [P, P], F32, kind="ExternalInput")
    ones = nc.dram_tensor("ones", [P, P], F32, kind="ExternalInput")
    y = nc.dram_tensor("y", [NB, C], F32, kind="ExternalOutput")
    io = (xT, wq, wk, wv, wo, kp, vp, gidx, cosq, sinq, cosk, sink,
          cmask, rperm, ident, ones, y)
    with nc.allow_low_precision(reason="float32r rounding for PE operands"):
        with tile.TileContext(nc) as tc:
            _emit(tc, io)
    nc.compile()
    return nc


def host_inputs(x, Wq, Wkv, Wo, K_pool, V_pool, slot_map, past_len):
    x = np.ascontiguousarray(np.asarray(x, dtype=np.float32))
    Wq = np.asarray(Wq, dtype=np.float32)
    Wkv = np.asarray(Wkv, dtype=np.float32)
    Wo = np.asarray(Wo, dtype=np.float32)
    K_pool = np.asarray(K_pool, dtype=np.float32)
    V_pool = np.asarray(V_pool, dtype=np.float32)
    slot_map = np.asarray(slot_map, dtype=np.int32)
    past = int(past_len)
    assert past == PAST, f"kernel hardcodes past_len={PAST}, got {past}"

    xT = np.ascontiguousarray(x.reshape(NB, C).T)

    # rope tables; argument arithmetic mirrors the f32 ops of the reference
    idx = np.arange(D // 2, dtype=np.float32)
    inv = np.float32(1.0) / np.float32(10000.0) ** (idx / np.float32(D // 2))
    inv = inv.astype(np.float32)
    t = np.arange(past, past + T, dtype=np.float32)
    freqs = (t[:, None] * inv[None, :]).astype(np.float32)
    emb = np.concatenate([freqs, freqs], axis=1)
    cos = np.cos(emb).astype(np.float32)
    sin = np.sin(emb).astype(np.float32)
    qscale = np.float32(1.0) / np.sqrt(np.float32(D))
    cosqT = np.ascontiguousarray((cos * qscale).T)
    sinqT = np.ascontiguousarray((sin * qscale).T)
    coskT = np.ascontiguousarray(cos.T)
    sinkT = np.ascontiguousarray(sin.T)

    s_i = np.arange(P, dtype=np.int64)[:, None]
    t_i = np.arange(TB, dtype=np.int64)[None, :]
    cm = np.empty((P, 4, TB), np.float32)
    for ri in range(4):
        cm[:, ri, :] = np.where(s_i <= t_i - ri * P, 0.0, NEG)

    gidx = slot_map[:, :past].reshape(B, 8, P).transpose(2, 0, 1).reshape(P, B * 8)
    gidx = np.ascontiguousarray(gidx.astype(np.int32))

    rperm = np.zeros((P, P), np.float32)
    for d in range(D // 2):
        rperm[d + D // 2, d] = -1.0       # rot(q)[d] = -q[d+64] for d < 64
        rperm[d, d + D // 2] = 1.0        # rot(q)[d] = q[d-64] for d >= 64
    ident = np.eye(P, dtype=np.float32)
    ones = np.ones((P, P), np.float32)

    in_maps = []
    for c in range(NCORES):
        in_maps.append({
            "xT": xT,
            "wq": np.ascontiguousarray(Wq[G * D * c:G * D * (c + 1), :].T),
            "wk": np.ascontiguousarray(Wkv[D * c:D * (c + 1), :].T),
            "wv": np.ascontiguousarray(Wkv[HKV * D + D * c:HKV * D + D * (c + 1), :].T),
            "wo": np.ascontiguousarray(Wo[:, G * D * c:G * D * (c + 1)].T),
            "kp": np.ascontiguousarray(K_pool[:, c, :]),
            "vp": np.ascontiguousarray(V_pool[:, c, :]),
            "gidx": gidx,
            "cosq": cosqT, "sinq": sinqT, "cosk": coskT, "sink": sinkT,
            "cmask": cm, "rperm": rperm, "ident": ident, "ones": ones,
        })
    return in_maps


_NC_CACHE = None


def kernel(**inputs):
    global _NC_CACHE
    in_maps = host_inputs(**inputs)
    if _NC_CACHE is None:
        _NC_CACHE = build_nc()
    res = run_bass_kernel_spmd(_NC_CACHE, in_maps, core_ids=list(range(NCORES)))
    y = res.results[0]["y"].astype(np.float32)
    for c in range(1, NCORES):
        y = y + res.results[c]["y"]
    return y.reshape(B, T, C)



# revision 4
# speedup vs baseline: 1.5314x; 1.5314x over previous
"""Trainium2 Bass kernel for paged causal self-attention (GQA + YaRN rope).

Sharding: tensor-parallel over heads. Core c (of 8) owns kv-head c and
q-heads 2c, 2c+1 for both batches. Each core computes a partial output
y_c = attn_c @ Wo_c.T over its 256 channels; the host sums the 8 partials.

The reference's scatter of new K/V into the pools is dead code w.r.t. the
returned output; new K/V are consumed directly from SBUF. The past-KV
gather (slot_map indexed) and the [s,d]/[d,s] layout transposes are done
on the host, so the device sees two contiguous fp16 layouts.

All matmul operands are fp16 (host-precast); accumulation stays fp32 in
PSUM. exp uses a constant -4 bias (cancels in softmax) for fp16 range
margin. Causal diagonal chunks are column-sliced and share one [128,128]
triangular mask tile.
"""

import sys

sys.path.insert(0, "/opt/trn_rl_repo")

import numpy as np

import concourse.bacc as bacc
import concourse.bass as bass
import concourse.tile as tile
from concourse import mybir
from concourse.bass_utils import run_bass_kernel_spmd

F32 = mybir.dt.float32
F16 = mybir.dt.float16
EXP = mybir.ActivationFunctionType.Exp

B, T, PAST = 2, 1024, 1024
H, HKV, D = 16, 8, 128
G = H // HKV            # q heads per kv head
C = H * D               # 2048
TOTAL = PAST + T        # 2048
NB = B * T              # 2048 flattened tokens
NCORES = 8
P = 128
TB = 512                # token block
NEG = -60000.0          # mask value (fp16-representable; exp underflows to 0)
EBIAS = -4.0            # constant exp bias; cancels in softmax


def _emit(tc, io):
    nc = tc.nc
    (xT, wq, wk, wv, wo, kpT, vpn, cosq, sinq, cosk, sink, tri, rperm,
     ones, y) = io

    with (
        tc.tile_pool(name="const", bufs=1) as cp,
        tc.tile_pool(name="persist", bufs=1) as pp,
        tc.tile_pool(name="ysb", bufs=4) as yp,
    ):
        # ---- constants / persistent weights (single contiguous DMAs) ----
        tri_t = cp.tile([P, P], F16)
        nc.sync.dma_start(tri_t[:], tri[:])
        rperm_t = cp.tile([P, P], F16)
        nc.sync.dma_start(rperm_t[:], rperm[:])
        ones_t = cp.tile([P, P], F16)
        nc.sync.dma_start(ones_t[:], ones[:])
        ebias = cp.tile([P, 1], F32)
        nc.vector.memset(ebias[:], EBIAS)

        kT_past = pp.tile([P, B, 8, P], F16)   # [d, b, chunk, s%128]
        nc.sync.dma_start(kT_past[:], kpT.rearrange("p (b j m) -> p b j m", b=B, j=8))
        vg = pp.tile([P, B, 8, P], F16)        # [s%128, b, chunk, d]
        nc.sync.dma_start(vg[:], vpn.rearrange("p (b j m) -> p b j m", b=B, j=8))

        wq_t = pp.tile([P, 16, G * P], F16)
        nc.sync.dma_start(wq_t[:], wq.rearrange("(kc p) m -> p kc m", p=P))
        wk_t = pp.tile([P, 16, P], F16)
        nc.sync.dma_start(wk_t[:], wk.rearrange("(kc p) m -> p kc m", p=P))
        wv_t = pp.tile([P, 16, P], F16)
        nc.sync.dma_start(wv_t[:], wv.rearrange("(kc p) m -> p kc m", p=P))
        wo_t = pp.tile([P, G, C], F16)
        nc.sync.dma_start(wo_t[:], wo.rearrange("(g p) m -> p g m", g=G))

        cosq_t = pp.tile([P, T], F16)
        nc.sync.dma_start(cosq_t[:], cosq[:])
        sinq_t = pp.tile([P, T], F16)
        nc.sync.dma_start(sinq_t[:], sinq[:])
        cosk_t = pp.tile([P, T], F16)
        nc.sync.dma_start(cosk_t[:], cosk[:])
        sink_t = pp.tile([P, T], F16)
        nc.sync.dma_start(sink_t[:], sink[:])

        # ---- persistent activations ----
        qT0 = pp.tile([P, NB], F16)       # q head 2c,   [d, token]
        qT1 = pp.tile([P, NB], F16)       # q head 2c+1
        kT_new = pp.tile([P, NB], F16)    # new keys,    [d, token]
        v_nat = pp.tile([P, B, 8, P], F16)     # new values, [t%128, b, chunk, d]
        att0 = pp.tile([P, NB], F16)      # attention out head 2c, [d, token]
        att1 = pp.tile([P, NB], F16)

        # ================= phase 1: projections + rope =================
        with (
            tc.tile_pool(name="xt", bufs=8) as xp,
            tc.tile_pool(name="rope", bufs=3) as rp,
            tc.tile_pool(name="pproj", bufs=1, space="PSUM") as pjp,
            tc.tile_pool(name="prope", bufs=2, space="PSUM") as rpp,
        ):
            for tb in range(NB // TB):           # 4 token blocks of 512
                n0 = tb * TB
                b = tb // 2
                tpos = (tb % 2) * TB             # position-in-batch of block

                q0p = pjp.tile([P, TB], F32, name="q0p", tag="q0")
                q1p = pjp.tile([P, TB], F32, name="q1p", tag="q1")
                kkp = pjp.tile([P, TB], F32, name="kkp", tag="kk")
                vvp = pjp.tile([P, TB], F32, name="vvp", tag="vv")
                for kc in range(16):
                    xt = xp.tile([P, TB], F16, name="xt", tag="xt")
                    nc.sync.dma_start(xt[:], xT[kc * P:(kc + 1) * P, n0:n0 + TB])
                    st = (kc == 0)
                    sp = (kc == 15)
                    nc.tensor.matmul(q0p[:], wq_t[:, kc, 0:P], xt[:], start=st, stop=sp)
                    nc.tensor.matmul(q1p[:], wq_t[:, kc, P:2 * P], xt[:], start=st, stop=sp)
                    nc.tensor.matmul(kkp[:], wk_t[:, kc, :], xt[:], start=st, stop=sp)
                    nc.tensor.matmul(vvp[:], wv_t[:, kc, :], xt[:], start=st, stop=sp)

                # rope for q0, q1, k  (dst = raw*cos + rot(raw)*sin)
                for src, dst, ct, stt in (
                    (q0p, qT0, cosq_t, sinq_t),
                    (q1p, qT1, cosq_t, sinq_t),
                    (kkp, kT_new, cosk_t, sink_t),
                ):
                    raw = rp.tile([P, TB], F16, name="raw", tag="raw")
                    nc.scalar.copy(raw[:], src[:])
                    rot = rpp.tile([P, TB], F32, name="rot", tag="rot")
                    nc.tensor.matmul(rot[:], rperm_t[:], raw[:], start=True, stop=True)
                    t1 = rp.tile([P, TB], F16, name="t1", tag="t1")
                    nc.vector.tensor_mul(t1[:], raw[:], ct[:, tpos:tpos + TB])
                    t2 = rp.tile([P, TB], F16, name="t2", tag="t2")
                    nc.vector.tensor_mul(t2[:], rot[:], stt[:, tpos:tpos + TB])
                    nc.vector.tensor_add(dst[:, n0:n0 + TB], t1[:], t2[:])

                # v: no rope; evacuate fp16 then DMA-transpose to [t, d]
                vsb = rp.tile([P, TB], F16, name="vsb", tag="vsb")
                nc.scalar.copy(vsb[:], vvp[:])
                for j4 in range(TB // P):
                    nc.sync.dma_start_transpose(
                        v_nat[:, b, (tb % 2) * 4 + j4, :],
                        vsb[:, j4 * P:(j4 + 1) * P])

        # ================= phase 2+3: attention + output proj =================
        with (
            tc.tile_pool(name="exps", bufs=2) as ep,
            tc.tile_pool(name="sums", bufs=2) as sp_,
            tc.tile_pool(name="attw", bufs=2) as aw,
            tc.tile_pool(name="pscore", bufs=2, space="PSUM") as scp,
            tc.tile_pool(name="pav", bufs=2, space="PSUM") as avp,
            tc.tile_pool(name="pbc", bufs=2, space="PSUM") as bcp,
            tc.tile_pool(name="py", bufs=2, space="PSUM") as pyp,
        ):
            for b in range(B):
                for tbq in range(2):             # query block of 512 in batch
                    t0 = b * T + tbq * TB
                    for g, (qT, att) in enumerate(((qT0, att0), (qT1, att1))):
                        q_ap = qT[:, t0:t0 + TB]
                        njnew = 4 * tbq + 4
                        nch = 8 + njnew

                        # chunk list: past (full), sliced-diag new (desc ri),
                        # then full new; last chunk is full-width.
                        chunks = [(kT_past[:, b, j, :], vg[:, b, j, :], None)
                                  for j in range(8)]
                        sliced = []
                        full_new = []
                        for j in range(njnew):
                            koff = b * T + j * P
                            ri = j - 4 * tbq
                            ent = (kT_new[:, koff:koff + P],
                                   v_nat[:, b, j, :], ri if ri > 0 else None,
                                   ri == 0)
                            if ri > 0:
                                sliced.append(ent)
                            else:
                                full_new.append(ent)
                        sliced.reverse()   # descending ri
                        # order: past, sliced (ri 3..1), full new (ri<0), ri==0 last
                        full_new.sort(key=lambda e: e[3])
                        chunks = ([(k_, v_, None, False) for k_, v_, _ in chunks]
                                  + sliced + full_new)

                        expS = ep.tile([P, 16, TB], F16, name="expS", tag="expS")
                        sumP = sp_.tile([P, TB], F16, name="sumP", tag="sumP")
                        av = avp.tile([P, TB], F32, name="av", tag="av")

                        for ci, (k_ap, v_ap, ri, diag0) in enumerate(chunks):
                            c0 = 0 if ri is None else P * ri
                            s_ps = scp.tile([P, TB], F32, name="s_ps", tag="s")
                            nc.tensor.matmul(s_ps[:, c0:], k_ap, q_ap[:, c0:],
                                             start=True, stop=True)
                            if ri is not None or diag0:
                                nc.vector.tensor_add(s_ps[:, c0:c0 + P],
                                                     s_ps[:, c0:c0 + P], tri_t[:])
                            e_ap = expS[:, ci, c0:]
                            nc.scalar.activation(e_ap, s_ps[:, c0:], EXP,
                                                 bias=ebias[:])
                            if ci == 0:
                                nc.vector.tensor_copy(sumP[:], e_ap)
                            else:
                                nc.vector.tensor_add(sumP[:, c0:], sumP[:, c0:], e_ap)
                            nc.tensor.matmul(av[:, c0:], v_ap, e_ap,
                                             start=(ci == 0), stop=(ci == nch - 1))

                        # softmax denominator: partition-reduce + broadcast via
                        # ones matmul, then fast approx reciprocal
                        rbc = bcp.tile([P, TB], F32, name="rbc", tag="rbc")
                        nc.tensor.matmul(rbc[:], ones_t[:], sumP[:],
                                         start=True, stop=True)
                        rinv = aw.tile([P, TB], F32, name="rinv", tag="rinv")
                        nc.vector.reciprocal_approx_fast(rinv[:], rbc[:])
                        nc.vector.tensor_mul(att[:, t0:t0 + TB], av[:], rinv[:])

                    # output projection for these 512 tokens
                    for tc4 in range(4):
                        tt0 = t0 + tc4 * P
                        for cb in range(4):
                            yps = pyp.tile([P, TB], F32, name="yps", tag="y")
                            nc.tensor.matmul(yps[:], att0[:, tt0:tt0 + P],
                                             wo_t[:, 0, cb * TB:(cb + 1) * TB],
                                             start=True, stop=False)
                            nc.tensor.matmul(yps[:], att1[:, tt0:tt0 + P],
                                             wo_t[:, 1, cb * TB:(cb + 1) * TB],
                                             start=False, stop=True)
                            ysb = yp.tile([P, TB], F16, name="ysbt", tag="ysbt")
                            if (tc4 + cb) % 2 == 0:
                                nc.scalar.copy(ysb[:], yps[:])
                            else:
                                nc.vector.tensor_copy(ysb[:], yps[:])
                            nc.sync.dma_start(
                                y[tt0:tt0 + P, cb * TB:(cb + 1) * TB], ysb[:])


def build_nc():
    nc = bacc.Bacc("TRN2")
    xT = nc.dram_tensor("xT", [C, NB], F16, kind="ExternalInput")
    wq = nc.dram_tensor("wq", [C, G * D], F16, kind="ExternalInput")
    wk = nc.dram_tensor("wk", [C, D], F16, kind="ExternalInput")
    wv = nc.dram_tensor("wv", [C, D], F16, kind="ExternalInput")
    wo = nc.dram_tensor("wo", [G * D, C], F16, kind="ExternalInput")
    kpT = nc.dram_tensor("kpT", [P, B * 8 * P], F16, kind="ExternalInput")
    vpn = nc.dram_tensor("vpn", [P, B * 8 * P], F16, kind="ExternalInput")
    cosq = nc.dram_tensor("cosq", [P, T], F16, kind="ExternalInput")
    sinq = nc.dram_tensor("sinq", [P, T], F16, kind="ExternalInput")
    cosk = nc.dram_tensor("cosk", [P, T], F16, kind="ExternalInput")
    sink = nc.dram_tensor("sink", [P, T], F16, kind="ExternalInput")
    tri = nc.dram_tensor("tri", [P, P], F16, kind="ExternalInput")
    rperm = nc.dram_tensor("rperm", [P, P], F16, kind="ExternalInput")
    ones = nc.dram_tensor("ones", [P, P], F16, kind="ExternalInput")
    y = nc.dram_tensor("y", [NB, C], F16, kind="ExternalOutput")
    io = (xT, wq, wk, wv, wo, kpT, vpn, cosq, sinq, cosk, sink, tri,
          rperm, ones, y)
    with nc.allow_low_precision(reason="fp16 operands; fp32 accumulation"):
        with tile.TileContext(nc) as tc:
            _emit(tc, io)
    nc.compile()
    return nc


def host_inputs(x, Wq, Wkv, Wo, K_pool, V_pool, slot_map, past_len):
    x = np.asarray(x, dtype=np.float32)
    Wq = np.asarray(Wq, dtype=np.float32)
    Wkv = np.asarray(Wkv, dtype=np.float32)
    Wo = np.asarray(Wo, dtype=np.float32)
    K_pool = np.asarray(K_pool, dtype=np.float32)
    V_pool = np.asarray(V_pool, dtype=np.float32)
    slot_map = np.asarray(slot_map, dtype=np.int32)
    past = int(past_len)
    assert past == PAST, f"kernel hardcodes past_len={PAST}, got {past}"

    xT = np.ascontiguousarray(x.reshape(NB, C).T.astype(np.float16))

    # rope tables; argument arithmetic mirrors the f32 ops of the reference
    idx = np.arange(D // 2, dtype=np.float32)
    inv = np.float32(1.0) / np.float32(10000.0) ** (idx / np.float32(D // 2))
    inv = inv.astype(np.float32)
    t = np.arange(past, past + T, dtype=np.float32)
    freqs = (t[:, None] * inv[None, :]).astype(np.float32)
    emb = np.concatenate([freqs, freqs], axis=1)
    cos = np.cos(emb).astype(np.float32)
    sin = np.sin(emb).astype(np.float32)
    qscale = np.float32(1.0) / np.sqrt(np.float32(D))
    cosqT = np.ascontiguousarray((cos * qscale).T.astype(np.float16))
    sinqT = np.ascontiguousarray((sin * qscale).T.astype(np.float16))
    coskT = np.ascontiguousarray(cos.T.astype(np.float16))
    sinkT = np.ascontiguousarray(sin.T.astype(np.float16))

    # shared [128,128] triangular mask for block-aligned causal diagonals
    s_i = np.arange(P)[:, None]
    u_i = np.arange(P)[None, :]
    tri = np.where(s_i <= u_i, 0.0, NEG).astype(np.float16)

    rperm = np.zeros((P, P), np.float32)
    for d in range(D // 2):
        rperm[d + D // 2, d] = -1.0       # rot(q)[d] = -q[d+64] for d < 64
        rperm[d, d + D // 2] = 1.0        # rot(q)[d] = q[d-64] for d >= 64
    rperm = rperm.astype(np.float16)
    ones = np.ones((P, P), np.float16)

    # host-side past-KV gather (+ transpose for K): logical past order
    gs = np.asarray(slot_map[:, :past], dtype=np.int64)     # [B, 1024]
    in_maps = []
    for c in range(NCORES):
        Kg = K_pool[gs, c, :].astype(np.float16)            # [B, 1024, 128]
        Vg = V_pool[gs, c, :].astype(np.float16)
        # kT_past [d, b, j, s%128]  -> flat [128, B*8*128]
        kpT = np.ascontiguousarray(
            Kg.reshape(B, 8, P, D).transpose(3, 0, 1, 2).reshape(P, B * 8 * P))
        # vg [s%128, b, j, d] -> flat [128, B*8*128]
        vpn = np.ascontiguousarray(
            Vg.reshape(B, 8, P, D).transpose(2, 0, 1, 3).reshape(P, B * 8 * P))
        in_maps.append({
            "xT": xT,
            "wq": np.ascontiguousarray(
                Wq[G * D * c:G * D * (c + 1), :].T.astype(np.float16)),
            "wk": np.ascontiguousarray(
                Wkv[D * c:D * (c + 1), :].T.astype(np.float16)),
            "wv": np.ascontiguousarray(
                Wkv[HKV * D + D * c:HKV * D + D * (c + 1), :].T.astype(np.float16)),
            "wo": np.ascontiguousarray(
                Wo[:, G * D * c:G * D * (c + 1)].T.astype(np.float16)),
            "kpT": kpT, "vpn": vpn,
            "cosq": cosqT, "sinq": sinqT, "cosk": coskT, "sink": sinkT,
            "tri": tri, "rperm": rperm, "ones": ones,
        })
    return in_maps


_NC_CACHE = None


def kernel(**inputs):
    global _NC_CACHE
    in_maps = host_inputs(**inputs)
    if _NC_CACHE is None:
        _NC_CACHE = build_nc()
    res = run_bass_kernel_spmd(_NC_CACHE, in_maps, core_ids=list(range(NCORES)))
    y = res.results[0]["y"].astype(np.float32)
    for c in range(1, NCORES):
        y = y + res.results[c]["y"].astype(np.float32)
    return y.reshape(B, T, C)


# revision 13
# speedup vs baseline: 1.5936x; 1.0406x over previous
"""Trainium2 Bass kernel for paged causal self-attention (GQA + YaRN rope).

Sharding: tensor-parallel over heads. Core c (of 8) owns kv-head c and
q-heads 2c, 2c+1 for both batches. Each core computes a partial output
y_c = attn_c @ Wo_c.T over its 256 channels; the host sums the 8 partials.

The reference's scatter of new K/V into the pools is dead code w.r.t. the
returned output; new K/V are consumed directly from SBUF. The past-KV
gather (slot_map indexed) and the [s,d]/[d,s] layout transposes are done
on the host, so the device sees two contiguous fp16 layouts.

All matmul operands are fp16 (host-precast); accumulation stays fp32 in
PSUM. exp uses a constant -4 bias (cancels in softmax) for fp16 range
margin. Causal diagonal chunks are column-sliced and share one [128,128]
triangular mask tile.
"""

import sys

sys.path.insert(0, "/opt/trn_rl_repo")

import ml_dtypes
import numpy as np

NP_BF16 = np.dtype(ml_dtypes.bfloat16)

import concourse.bacc as bacc
import concourse.bass as bass
import concourse.tile as tile
from concourse import mybir
from concourse.bass_utils import run_bass_kernel_spmd

F32 = mybir.dt.float32
F16 = mybir.dt.float16
BF16 = mybir.dt.bfloat16
# stationary matmul operands use bf16 (enables fast weight load);
# moving operands stay fp16 for precision
STAT = BF16
EXP = mybir.ActivationFunctionType.Exp

B, T, PAST = 2, 1024, 1024
H, HKV, D = 16, 8, 128
G = H // HKV            # q heads per kv head
C = H * D               # 2048
TOTAL = PAST + T        # 2048
NB = B * T              # 2048 flattened tokens
NCORES = 8
P = 128
TB = 512                # token block
NEG = -60000.0          # mask value (fp16-representable; exp underflows to 0)
EBIAS = -4.0            # constant exp bias; cancels in softmax


def _emit(tc, io):
    nc = tc.nc
    (xT, wq, wk, wv, wo, kpT, vpn, cosq, sinq, cosk, sink, tri, rperm,
     ones, y) = io

    with (
        tc.tile_pool(name="const", bufs=1) as cp,
        tc.tile_pool(name="persist", bufs=1) as pp,
        tc.tile_pool(name="ysb", bufs=4) as yp,
    ):
        # ---- weights needed first: sync queue (ahead of the x stream) ----
        wq_t = pp.tile([P, 16, G * P], STAT)
        nc.sync.dma_start(wq_t[:], wq.rearrange("(kc p) m -> p kc m", p=P))
        wk_t = pp.tile([P, 16, P], STAT)
        nc.sync.dma_start(wk_t[:], wk.rearrange("(kc p) m -> p kc m", p=P))
        wv_t = pp.tile([P, 16, P], STAT)
        nc.sync.dma_start(wv_t[:], wv.rearrange("(kc p) m -> p kc m", p=P))

        # ---- everything else: scalar HWDGE queue (doesn't block x stream) ----
        tri_t = cp.tile([P, P], F16)
        nc.scalar.dma_start(tri_t[:], tri[:])
        rperm_t = cp.tile([P, P], STAT)
        nc.scalar.dma_start(rperm_t[:], rperm[:])
        ones_t = cp.tile([P, P], STAT)
        nc.scalar.dma_start(ones_t[:], ones[:])
        ebias = cp.tile([P, 1], F32)
        nc.vector.memset(ebias[:], EBIAS)

        cosq_t = pp.tile([P, T], F16)
        nc.scalar.dma_start(cosq_t[:], cosq[:])
        sinq_t = pp.tile([P, T], F16)
        nc.scalar.dma_start(sinq_t[:], sinq[:])
        cosk_t = pp.tile([P, T], F16)
        nc.scalar.dma_start(cosk_t[:], cosk[:])
        sink_t = pp.tile([P, T], F16)
        nc.scalar.dma_start(sink_t[:], sink[:])

        kT_past = pp.tile([P, B, 8, P], STAT)  # [d, b, chunk, s%128]
        nc.scalar.dma_start(kT_past[:], kpT.rearrange("p (b j m) -> p b j m", b=B, j=8))
        vg = pp.tile([P, B, 8, P], STAT)       # [s%128, b, chunk, d]
        nc.scalar.dma_start(vg[:], vpn.rearrange("p (b j m) -> p b j m", b=B, j=8))
        wo_t = pp.tile([P, G, C], F16)
        nc.scalar.dma_start(wo_t[:], wo.rearrange("(g p) m -> p g m", g=G))

        # ---- persistent activations ----
        qT0 = pp.tile([P, NB], F16)       # q head 2c,   [d, token]  (moving)
        qT1 = pp.tile([P, NB], F16)       # q head 2c+1
        kT_new = pp.tile([P, NB], STAT)   # new keys,    [d, token]  (stationary)
        v_nat = pp.tile([P, B, 8, P], STAT)    # new values, [t%128, b, chunk, d]
        att0 = pp.tile([P, NB], STAT)     # attention out head 2c, [d, token]
        att1 = pp.tile([P, NB], STAT)

        # ================= phase 1: projections + rope =================
        with (
            tc.tile_pool(name="xt", bufs=12) as xp,
            tc.tile_pool(name="rope", bufs=3) as rp,
            tc.tile_pool(name="pproj", bufs=1, space="PSUM") as pjp,
            tc.tile_pool(name="prope", bufs=2, space="PSUM") as rpp,
        ):
            for tb in range(NB // TB):           # 4 token blocks of 512
                n0 = tb * TB
                b = tb // 2
                tpos = (tb % 2) * TB             # position-in-batch of block

                q0p = pjp.tile([P, TB], F32, name="q0p", tag="q0", bufs=2)
                q1p = pjp.tile([P, TB], F32, name="q1p", tag="q1", bufs=2)
                kkp = pjp.tile([P, TB], F32, name="kkp", tag="kk")
                vvp = pjp.tile([P, TB], F32, name="vvp", tag="vv")
                for kc in range(16):
                    xt = xp.tile([P, TB], F16, name="xt", tag="xt")
                    nc.sync.dma_start(xt[:], xT[kc * P:(kc + 1) * P, n0:n0 + TB])
                    st = (kc == 0)
                    sp = (kc == 15)
                    nc.tensor.matmul(q0p[:], wq_t[:, kc, 0:P], xt[:], start=st, stop=sp)
                    nc.tensor.matmul(q1p[:], wq_t[:, kc, P:2 * P], xt[:], start=st, stop=sp)
                    nc.tensor.matmul(kkp[:], wk_t[:, kc, :], xt[:], start=st, stop=sp)
                    nc.tensor.matmul(vvp[:], wv_t[:, kc, :], xt[:], start=st, stop=sp)

                # evacuate the single-buffered kk/vv banks first, on separate
                # engines, so the next block's matmuls aren't blocked
                raw_k = rp.tile([P, TB], F16, name="raw_k", tag="rawk")
                nc.scalar.copy(raw_k[:], kkp[:])
                vsb = rp.tile([P, TB], STAT, name="vsb", tag="vsb")
                nc.vector.tensor_copy(vsb[:], vvp[:])
                for j4 in range(TB // P):
                    nc.scalar.dma_start_transpose(
                        v_nat[:, b, (tb % 2) * 4 + j4, :],
                        vsb[:, j4 * P:(j4 + 1) * P])

                # rope (dst = raw*cos + rot(raw)*sin); k first
                for src, raw, dst, ct, stt in (
                    (None, raw_k, kT_new, cosk_t, sink_t),
                    (q0p, None, qT0, cosq_t, sinq_t),
                    (q1p, None, qT1, cosq_t, sinq_t),
                ):
                    if raw is None:
                        raw = rp.tile([P, TB], F16, name="raw", tag="raw")
                        nc.scalar.copy(raw[:], src[:])
                    rot = rpp.tile([P, TB], F32, name="rot", tag="rot")
                    nc.tensor.matmul(rot[:], rperm_t[:], raw[:], start=True, stop=True)
                    t1 = rp.tile([P, TB], F16, name="t1", tag="t1")
                    nc.vector.tensor_mul(t1[:], raw[:], ct[:, tpos:tpos + TB])
                    t2 = rp.tile([P, TB], F16, name="t2", tag="t2")
                    nc.vector.tensor_mul(t2[:], rot[:], stt[:, tpos:tpos + TB])
                    nc.vector.tensor_add(dst[:, n0:n0 + TB], t1[:], t2[:])

        # ================= phase 2+3: attention + output proj =================
        with (
            tc.tile_pool(name="exps", bufs=2) as ep,
            tc.tile_pool(name="sums", bufs=2) as sp_,
            tc.tile_pool(name="attw", bufs=2) as aw,
            tc.tile_pool(name="pscore", bufs=2, space="PSUM") as scp,
            tc.tile_pool(name="pav", bufs=2, space="PSUM") as avp,
            tc.tile_pool(name="pbc", bufs=2, space="PSUM") as bcp,
            tc.tile_pool(name="py", bufs=2, space="PSUM") as pyp,
        ):
            for b in range(B):
                for tbq in range(2):             # query block of 512 in batch
                    t0 = b * T + tbq * TB
                    for g, (qT, att) in enumerate(((qT0, att0), (qT1, att1))):
                        q_ap = qT[:, t0:t0 + TB]
                        njnew = 4 * tbq + 4
                        nch = 8 + njnew

                        # chunk list: past (full), sliced-diag new (desc ri),
                        # then full new; last chunk is full-width.
                        chunks = [(kT_past[:, b, j, :], vg[:, b, j, :], None)
                                  for j in range(8)]
                        sliced = []
                        full_new = []
                        for j in range(njnew):
                            koff = b * T + j * P
                            ri = j - 4 * tbq
                            ent = (kT_new[:, koff:koff + P],
                                   v_nat[:, b, j, :], ri if ri > 0 else None,
                                   ri == 0)
                            if ri > 0:
                                sliced.append(ent)
                            else:
                                full_new.append(ent)
                        sliced.reverse()   # descending ri
                        # order: past, sliced (ri 3..1), full new (ri<0), ri==0 last
                        full_new.sort(key=lambda e: e[3])
                        chunks = ([(k_, v_, None, False) for k_, v_, _ in chunks]
                                  + sliced + full_new)

                        expS = ep.tile([P, 16, TB], F16, name="expS", tag="expS")
                        sumP = sp_.tile([P, TB], F16, name="sumP", tag="sumP")
                        av = avp.tile([P, TB], F32, name="av", tag="av")

                        for ci, (k_ap, v_ap, ri, diag0) in enumerate(chunks):
                            c0 = 0 if ri is None else P * ri
                            s_ps = scp.tile([P, TB], F32, name="s_ps", tag="s")
                            nc.tensor.matmul(s_ps[:, c0:], k_ap, q_ap[:, c0:],
                                             start=True, stop=True)
                            if ri is not None or diag0:
                                nc.vector.tensor_add(s_ps[:, c0:c0 + P],
                                                     s_ps[:, c0:c0 + P], tri_t[:])
                            e_ap = expS[:, ci, c0:]
                            nc.scalar.activation(e_ap, s_ps[:, c0:], EXP,
                                                 bias=ebias[:])
                            if ci == 0:
                                nc.vector.tensor_copy(sumP[:], e_ap)
                            else:
                                nc.vector.tensor_add(sumP[:, c0:], sumP[:, c0:], e_ap)
                            nc.tensor.matmul(av[:, c0:], v_ap, e_ap,
                                             start=(ci == 0), stop=(ci == nch - 1))

                        # softmax denominator: partition-reduce + broadcast via
                        # ones matmul, then fast approx reciprocal
                        rbc = bcp.tile([P, TB], F32, name="rbc", tag="rbc")
                        nc.tensor.matmul(rbc[:], ones_t[:], sumP[:],
                                         start=True, stop=True)
                        rinv = aw.tile([P, TB], F32, name="rinv", tag="rinv")
                        nc.vector.reciprocal_approx_fast(rinv[:], rbc[:])
                        nc.vector.tensor_mul(att[:, t0:t0 + TB], av[:], rinv[:])

                    # output projection for these 512 tokens
                    for tc4 in range(4):
                        tt0 = t0 + tc4 * P
                        for cb in range(4):
                            yps = pyp.tile([P, TB], F32, name="yps", tag="y")
                            nc.tensor.matmul(yps[:], att0[:, tt0:tt0 + P],
                                             wo_t[:, 0, cb * TB:(cb + 1) * TB],
                                             start=True, stop=False)
                            nc.tensor.matmul(yps[:], att1[:, tt0:tt0 + P],
                                             wo_t[:, 1, cb * TB:(cb + 1) * TB],
                                             start=False, stop=True)
                            ysb = yp.tile([P, TB], F16, name="ysbt", tag="ysbt")
                            if (tc4 + cb) % 2 == 0:
                                nc.scalar.copy(ysb[:], yps[:])
                            else:
                                nc.vector.tensor_copy(ysb[:], yps[:])
                            nc.sync.dma_start(
                                y[tt0:tt0 + P, cb * TB:(cb + 1) * TB], ysb[:])


def build_nc():
    nc = bacc.Bacc("TRN2")
    xT = nc.dram_tensor("xT", [C, NB], F16, kind="ExternalInput")
    wq = nc.dram_tensor("wq", [C, G * D], STAT, kind="ExternalInput")
    wk = nc.dram_tensor("wk", [C, D], STAT, kind="ExternalInput")
    wv = nc.dram_tensor("wv", [C, D], STAT, kind="ExternalInput")
    wo = nc.dram_tensor("wo", [G * D, C], F16, kind="ExternalInput")
    kpT = nc.dram_tensor("kpT", [P, B * 8 * P], STAT, kind="ExternalInput")
    vpn = nc.dram_tensor("vpn", [P, B * 8 * P], STAT, kind="ExternalInput")
    cosq = nc.dram_tensor("cosq", [P, T], F16, kind="ExternalInput")
    sinq = nc.dram_tensor("sinq", [P, T], F16, kind="ExternalInput")
    cosk = nc.dram_tensor("cosk", [P, T], F16, kind="ExternalInput")
    sink = nc.dram_tensor("sink", [P, T], F16, kind="ExternalInput")
    tri = nc.dram_tensor("tri", [P, P], F16, kind="ExternalInput")
    rperm = nc.dram_tensor("rperm", [P, P], STAT, kind="ExternalInput")
    ones = nc.dram_tensor("ones", [P, P], STAT, kind="ExternalInput")
    y = nc.dram_tensor("y", [NB, C], F16, kind="ExternalOutput")
    io = (xT, wq, wk, wv, wo, kpT, vpn, cosq, sinq, cosk, sink, tri,
          rperm, ones, y)
    with nc.allow_low_precision(reason="fp16 operands; fp32 accumulation"):
        with tile.TileContext(nc) as tc:
            _emit(tc, io)
    nc.compile()
    return nc


def host_inputs(x, Wq, Wkv, Wo, K_pool, V_pool, slot_map, past_len):
    x = np.asarray(x, dtype=np.float32)
    Wq = np.asarray(Wq, dtype=np.float32)
    Wkv = np.asarray(Wkv, dtype=np.float32)
    Wo = np.asarray(Wo, dtype=np.float32)
    K_pool = np.asarray(K_pool, dtype=np.float32)
    V_pool = np.asarray(V_pool, dtype=np.float32)
    slot_map = np.asarray(slot_map, dtype=np.int32)
    past = int(past_len)
    assert past == PAST, f"kernel hardcodes past_len={PAST}, got {past}"

    xT = np.ascontiguousarray(x.reshape(NB, C).T.astype(np.float16))

    # rope tables; argument arithmetic mirrors the f32 ops of the reference
    idx = np.arange(D // 2, dtype=np.float32)
    inv = np.float32(1.0) / np.float32(10000.0) ** (idx / np.float32(D // 2))
    inv = inv.astype(np.float32)
    t = np.arange(past, past + T, dtype=np.float32)
    freqs = (t[:, None] * inv[None, :]).astype(np.float32)
    emb = np.concatenate([freqs, freqs], axis=1)
    cos = np.cos(emb).astype(np.float32)
    sin = np.sin(emb).astype(np.float32)
    qscale = np.float32(1.0) / np.sqrt(np.float32(D))
    cosqT = np.ascontiguousarray((cos * qscale).T.astype(np.float16))
    sinqT = np.ascontiguousarray((sin * qscale).T.astype(np.float16))
    coskT = np.ascontiguousarray(cos.T.astype(np.float16))
    sinkT = np.ascontiguousarray(sin.T.astype(np.float16))

    # shared [128,128] triangular mask for block-aligned causal diagonals
    s_i = np.arange(P)[:, None]
    u_i = np.arange(P)[None, :]
    tri = np.where(s_i <= u_i, 0.0, NEG).astype(np.float16)

    rperm = np.zeros((P, P), np.float32)
    for d in range(D // 2):
        rperm[d + D // 2, d] = -1.0       # rot(q)[d] = -q[d+64] for d < 64
        rperm[d, d + D // 2] = 1.0        # rot(q)[d] = q[d-64] for d >= 64
    rperm = rperm.astype(NP_BF16)
    ones = np.ones((P, P), NP_BF16)

    # host-side past-KV gather (+ transpose for K): logical past order
    gs = np.asarray(slot_map[:, :past], dtype=np.int64)     # [B, 1024]
    in_maps = []
    for c in range(NCORES):
        Kg = K_pool[gs, c, :].astype(NP_BF16)               # [B, 1024, 128]
        Vg = V_pool[gs, c, :].astype(NP_BF16)
        # kT_past [d, b, j, s%128]  -> flat [128, B*8*128]
        kpT = np.ascontiguousarray(
            Kg.reshape(B, 8, P, D).transpose(3, 0, 1, 2).reshape(P, B * 8 * P))
        # vg [s%128, b, j, d] -> flat [128, B*8*128]
        vpn = np.ascontiguousarray(
            Vg.reshape(B, 8, P, D).transpose(2, 0, 1, 3).reshape(P, B * 8 * P))
        in_maps.append({
            "xT": xT,
            "wq": np.ascontiguousarray(
                Wq[G * D * c:G * D * (c + 1), :].T.astype(NP_BF16)),
            "wk": np.ascontiguousarray(
                Wkv[D * c:D * (c + 1), :].T.astype(NP_BF16)),
            "wv": np.ascontiguousarray(
                Wkv[HKV * D + D * c:HKV * D + D * (c + 1), :].T.astype(NP_BF16)),
            "wo": np.ascontiguousarray(
                Wo[:, G * D * c:G * D * (c + 1)].T.astype(np.float16)),
            "kpT": kpT, "vpn": vpn,
            "cosq": cosqT, "sinq": sinqT, "cosk": coskT, "sink": sinkT,
            "tri": tri, "rperm": rperm, "ones": ones,
        })
    return in_maps


_NC_CACHE = None


def kernel(**inputs):
    global _NC_CACHE
    in_maps = host_inputs(**inputs)
    if _NC_CACHE is None:
        _NC_CACHE = build_nc()
    res = run_bass_kernel_spmd(_NC_CACHE, in_maps, core_ids=list(range(NCORES)))
    y = res.results[0]["y"].astype(np.float32)
    for c in range(1, NCORES):
        y = y + res.results[c]["y"].astype(np.float32)
    return y.reshape(B, T, C)


# revision 23
# speedup vs baseline: 1.7268x; 1.0836x over previous
"""Trainium2 Bass kernel for paged causal self-attention (GQA + YaRN rope).

Sharding: tensor-parallel over heads. Core c (of 8) owns kv-head c and
q-heads 2c, 2c+1 for both batches. Each core computes a partial output
y_c = attn_c @ Wo_c.T over its 256 channels; the host sums the 8 partials.

The reference's scatter of new K/V into the pools is dead code w.r.t. the
returned output; new K/V are consumed directly from SBUF. The past-KV
gather (slot_map indexed) and the [s,d]/[d,s] layout transposes are done
on the host, so the device sees two contiguous fp16 layouts.

All matmul operands are fp16 (host-precast); accumulation stays fp32 in
PSUM. exp uses a constant -4 bias (cancels in softmax) for fp16 range
margin. Causal diagonal chunks are column-sliced and share one [128,128]
triangular mask tile.
"""

import sys

sys.path.insert(0, "/opt/trn_rl_repo")

import ml_dtypes
import numpy as np

NP_BF16 = np.dtype(ml_dtypes.bfloat16)

import concourse.bacc as bacc
import concourse.bass as bass
import concourse.tile as tile
from concourse import mybir
from concourse.bass_utils import run_bass_kernel_spmd

F32 = mybir.dt.float32
F16 = mybir.dt.float16
BF16 = mybir.dt.bfloat16
# stationary matmul operands use bf16 (enables fast weight load);
# moving operands stay fp16 for precision
STAT = BF16
EXP = mybir.ActivationFunctionType.Exp

B, T, PAST = 2, 1024, 1024
H, HKV, D = 16, 8, 128
G = H // HKV            # q heads per kv head
C = H * D               # 2048
TOTAL = PAST + T        # 2048
NB = B * T              # 2048 flattened tokens
NCORES = 8
P = 128
TB = 512                # token block
NEG = -60000.0          # mask value (fp16-representable; exp underflows to 0)
EBIAS = -4.0            # constant exp bias; cancels in softmax


def _emit(tc, io):
    nc = tc.nc
    (xT, wq, wk, wv, wo, kpT, vpn, cosq, sinq, cosk, sink, tri, rperm,
     ones, ident, y) = io

    with (
        tc.tile_pool(name="const", bufs=1) as cp,
        tc.tile_pool(name="persist", bufs=1) as pp,
        tc.tile_pool(name="ysb", bufs=4) as yp,
    ):
        # ---- weights needed first: sync queue (ahead of the x stream);
        # host pre-arranges so each load is contiguous per partition ----
        wq_t = pp.tile([P, 16, G * P], STAT)
        nc.sync.dma_start(wq_t[:], wq.rearrange("p (kc m) -> p kc m", kc=16))
        wk_t = pp.tile([P, 16, P], STAT)
        nc.sync.dma_start(wk_t[:], wk.rearrange("p (kc m) -> p kc m", kc=16))
        wv_t = pp.tile([P, 16, P], STAT)
        nc.sync.dma_start(wv_t[:], wv.rearrange("p (kc m) -> p kc m", kc=16))

        # ---- everything else: scalar HWDGE queue (doesn't block x stream) ----
        tri_t = cp.tile([P, P], F16)
        nc.scalar.dma_start(tri_t[:], tri[:])
        rperm_t = cp.tile([P, P], STAT)
        nc.scalar.dma_start(rperm_t[:], rperm[:])
        ones_t = cp.tile([P, P], STAT)
        nc.scalar.dma_start(ones_t[:], ones[:])
        ident_t = cp.tile([P, P], STAT)
        nc.scalar.dma_start(ident_t[:], ident[:])
        ebias = cp.tile([P, 1], F32)
        nc.vector.memset(ebias[:], EBIAS)

        cosq_t = pp.tile([P, T], F16)
        nc.scalar.dma_start(cosq_t[:], cosq[:])
        sinq_t = pp.tile([P, T], F16)
        nc.scalar.dma_start(sinq_t[:], sinq[:])
        cosk_t = pp.tile([P, T], F16)
        nc.scalar.dma_start(cosk_t[:], cosk[:])
        sink_t = pp.tile([P, T], F16)
        nc.scalar.dma_start(sink_t[:], sink[:])

        kT_past = pp.tile([P, B, 8, P], STAT)  # [d, b, chunk, s%128]
        nc.scalar.dma_start(kT_past[:], kpT.rearrange("p (b j m) -> p b j m", b=B, j=8))
        vg = pp.tile([P, B, 8, P], STAT)       # [s%128, b, chunk, d]
        nc.scalar.dma_start(vg[:], vpn.rearrange("p (b j m) -> p b j m", b=B, j=8))
        wo_t = pp.tile([P, G, C], F16)
        nc.scalar.dma_start(wo_t[:], wo.rearrange("p (g m) -> p g m", g=G))

        # ---- persistent activations ----
        qT0 = pp.tile([P, NB], F16)       # q head 2c,   [d, token]  (moving)
        qT1 = pp.tile([P, NB], F16)       # q head 2c+1
        kT_new = pp.tile([P, NB], STAT)   # new keys,    [d, token]  (stationary)
        v_nat = pp.tile([P, B, 8, P], STAT)    # new values, [t%128, b, chunk, d]
        att0 = pp.tile([P, NB], STAT)     # attention out head 2c, [d, token]
        att1 = pp.tile([P, NB], STAT)

        # ================= phase 1: projections + rope =================
        with (
            tc.tile_pool(name="xt", bufs=12) as xp,
            tc.tile_pool(name="rope", bufs=3) as rp,
            tc.tile_pool(name="pproj", bufs=1, space="PSUM") as pjp,
            tc.tile_pool(name="prope", bufs=2, space="PSUM") as rpp,
            tc.tile_pool(name="ptr", bufs=2, space="PSUM") as trp,
        ):
            # Rope/v-transpose emission is deferred one block: the PE's
            # in-order stream gets block N+1's projection matmuls BEFORE
            # block N's rope matmuls, so PSUM-evacuation copies (ACT/DVE)
            # have a full block of slack to land.
            pending = None

            def emit_rope(state):
                tbp, raw_k_, raw_q0_, raw_q1_, vsb_ = state
                n0p = tbp * TB
                bp = tbp // 2
                tposp = (tbp % 2) * TB
                # v transposes first (their input was evacuated earliest);
                # all four land in one PSUM tile, one batched evacuation
                vtp = trp.tile([P, 4, P], STAT, name="vtp", tag="vtp")
                for j4 in range(TB // P):
                    nc.tensor.transpose(vtp[:, j4, :],
                                        vsb_[:, j4 * P:(j4 + 1) * P], ident_t[:])
                for raw, dst, ct, stt in (
                    (raw_k_, kT_new, cosk_t, sink_t),
                    (raw_q0_, qT0, cosq_t, sinq_t),
                    (raw_q1_, qT1, cosq_t, sinq_t),
                ):
                    rot = rpp.tile([P, TB], F32, name="rot", tag="rot")
                    nc.tensor.matmul(rot[:], rperm_t[:], raw[:], start=True,
                                     stop=True)
                    t1 = rp.tile([P, TB], F16, name="t1", tag="t1")
                    nc.vector.tensor_mul(t1[:], raw[:], ct[:, tposp:tposp + TB])
                    t2 = rp.tile([P, TB], F16, name="t2", tag="t2")
                    nc.vector.tensor_mul(t2[:], rot[:], stt[:, tposp:tposp + TB])
                    nc.vector.tensor_add(dst[:, n0p:n0p + TB], t1[:], t2[:])
                nc.vector.tensor_copy(
                    v_nat[:, bp, (tbp % 2) * 4:(tbp % 2) * 4 + 4, :], vtp[:])

            for tb in range(NB // TB):           # 4 token blocks of 512
                n0 = tb * TB

                # previous block's q psum evacuations (ACT queue, ready now)
                if pending is not None:
                    tbp, q0p_, q1p_, raw_k_, vsb_ = pending
                    raw_q0 = rp.tile([P, TB], F16, name="raw_q0", tag="rawq0")
                    nc.scalar.copy(raw_q0[:], q0p_[:])
                    raw_q1 = rp.tile([P, TB], F16, name="raw_q1", tag="rawq1")
                    nc.scalar.copy(raw_q1[:], q1p_[:])
                    pending = (tbp, raw_k_, raw_q0, raw_q1, vsb_)

                q0p = pjp.tile([P, TB], F32, name="q0p", tag="q0")
                q1p = pjp.tile([P, TB], F32, name="q1p", tag="q1")
                kkp = pjp.tile([P, TB], F32, name="kkp", tag="kk")
                vvp = pjp.tile([P, TB], F32, name="vvp", tag="vv")
                for kc in range(16):
                    xt = xp.tile([P, TB], F16, name="xt", tag="xt")
                    nc.sync.dma_start(xt[:], xT[kc * P:(kc + 1) * P, n0:n0 + TB])
                    st = (kc == 0)
                    sp = (kc == 15)
                    nc.tensor.matmul(q0p[:], wq_t[:, kc, 0:P], xt[:], start=st, stop=sp)
                    nc.tensor.matmul(q1p[:], wq_t[:, kc, P:2 * P], xt[:], start=st, stop=sp)
                    nc.tensor.matmul(kkp[:], wk_t[:, kc, :], xt[:], start=st, stop=sp)
                    nc.tensor.matmul(vvp[:], wv_t[:, kc, :], xt[:], start=st, stop=sp)

                # evacuate kk/vv immediately on separate engines
                raw_k = rp.tile([P, TB], F16, name="raw_k", tag="rawk")
                nc.scalar.copy(raw_k[:], kkp[:])
                vsb = rp.tile([P, TB], STAT, name="vsb", tag="vsb")
                nc.vector.tensor_copy(vsb[:], vvp[:])

                if pending is not None:
                    emit_rope(pending)
                pending = (tb, q0p, q1p, raw_k, vsb)

            # drain the final block
            tbp, q0p_, q1p_, raw_k_, vsb_ = pending
            raw_q0 = rp.tile([P, TB], F16, name="raw_q0", tag="rawq0")
            nc.scalar.copy(raw_q0[:], q0p_[:])
            raw_q1 = rp.tile([P, TB], F16, name="raw_q1", tag="rawq1")
            nc.scalar.copy(raw_q1[:], q1p_[:])
            emit_rope((tbp, raw_k_, raw_q0, raw_q1, vsb_))

        # ================= phase 2+3: attention + output proj =================
        with (
            tc.tile_pool(name="exps", bufs=2) as ep,
            tc.tile_pool(name="sums", bufs=2) as sp_,
            tc.tile_pool(name="attw", bufs=2) as aw,
            tc.tile_pool(name="pscore", bufs=2, space="PSUM") as scp,
            tc.tile_pool(name="pav", bufs=2, space="PSUM") as avp,
            tc.tile_pool(name="pbc", bufs=2, space="PSUM") as bcp,
            tc.tile_pool(name="py", bufs=2, space="PSUM") as pyp,
        ):
            for b in range(B):
                for tbq in range(2):             # query block of 512 in batch
                    t0 = b * T + tbq * TB
                    for g, (qT, att) in enumerate(((qT0, att0), (qT1, att1))):
                        q_ap = qT[:, t0:t0 + TB]
                        njnew = 4 * tbq + 4
                        nch = 8 + njnew

                        # chunk list: past (full), sliced-diag new (desc ri),
                        # then full new; last chunk is full-width.
                        chunks = [(kT_past[:, b, j, :], vg[:, b, j, :], None)
                                  for j in range(8)]
                        sliced = []
                        full_new = []
                        for j in range(njnew):
                            koff = b * T + j * P
                            ri = j - 4 * tbq
                            ent = (kT_new[:, koff:koff + P],
                                   v_nat[:, b, j, :], ri if ri > 0 else None,
                                   ri == 0)
                            if ri > 0:
                                sliced.append(ent)
                            else:
                                full_new.append(ent)
                        sliced.reverse()   # descending ri
                        # order: past, sliced (ri 3..1), full new (ri<0), ri==0 last
                        full_new.sort(key=lambda e: e[3])
                        chunks = ([(k_, v_, None, False) for k_, v_, _ in chunks]
                                  + sliced + full_new)

                        expS = ep.tile([P, 16, TB], F16, name="expS", tag="expS")
                        sumP = sp_.tile([P, TB], F16, name="sumP", tag="sumP")
                        av = avp.tile([P, TB], F32, name="av", tag="av")

                        # scores run one chunk ahead of av on the PE so the
                        # exp (ACT) latency of chunk ci hides under the
                        # score matmul of chunk ci+1
                        pend_av = None
                        for ci, (k_ap, v_ap, ri, diag0) in enumerate(chunks):
                            c0 = 0 if ri is None else P * ri
                            s_ps = scp.tile([P, TB], F32, name="s_ps", tag="s")
                            nc.tensor.matmul(s_ps[:, c0:], k_ap, q_ap[:, c0:],
                                             start=True, stop=True)
                            if ri is not None or diag0:
                                nc.vector.tensor_add(s_ps[:, c0:c0 + P],
                                                     s_ps[:, c0:c0 + P], tri_t[:])
                            e_ap = expS[:, ci, c0:]
                            nc.scalar.activation(e_ap, s_ps[:, c0:], EXP,
                                                 bias=ebias[:])
                            if ci == 0:
                                nc.vector.tensor_copy(sumP[:], e_ap)
                            else:
                                nc.vector.tensor_add(sumP[:, c0:], sumP[:, c0:],
                                                     e_ap)
                            if pend_av is not None:
                                pv_ap, pe_ap, pc0, pci = pend_av
                                nc.tensor.matmul(av[:, pc0:], pv_ap, pe_ap,
                                                 start=(pci == 0), stop=False)
                            pend_av = (v_ap, e_ap, c0, ci)
                        pv_ap, pe_ap, pc0, pci = pend_av
                        nc.tensor.matmul(av[:, pc0:], pv_ap, pe_ap,
                                         start=False, stop=True)

                        # softmax denominator: partition-reduce + broadcast via
                        # ones matmul, then fast approx reciprocal
                        rbc = bcp.tile([P, TB], F32, name="rbc", tag="rbc")
                        nc.tensor.matmul(rbc[:], ones_t[:], sumP[:],
                                         start=True, stop=True)
                        rinv = aw.tile([P, TB], F32, name="rinv", tag="rinv")
                        nc.vector.reciprocal_approx_fast(rinv[:], rbc[:])
                        nc.vector.tensor_mul(att[:, t0:t0 + TB], av[:], rinv[:])

                    # output projection for these 512 tokens
                    for tc4 in range(4):
                        tt0 = t0 + tc4 * P
                        for cb in range(4):
                            yps = pyp.tile([P, TB], F32, name="yps", tag="y")
                            nc.tensor.matmul(yps[:], att0[:, tt0:tt0 + P],
                                             wo_t[:, 0, cb * TB:(cb + 1) * TB],
                                             start=True, stop=False)
                            nc.tensor.matmul(yps[:], att1[:, tt0:tt0 + P],
                                             wo_t[:, 1, cb * TB:(cb + 1) * TB],
                                             start=False, stop=True)
                            ysb = yp.tile([P, TB], F16, name="ysbt", tag="ysbt")
                            if (tc4 + cb) % 2 == 0:
                                nc.scalar.copy(ysb[:], yps[:])
                            else:
                                nc.vector.tensor_copy(ysb[:], yps[:])
                            nc.sync.dma_start(
                                y[tt0:tt0 + P, cb * TB:(cb + 1) * TB], ysb[:])


def build_nc():
    nc = bacc.Bacc("TRN2")
    xT = nc.dram_tensor("xT", [C, NB], F16, kind="ExternalInput")
    wq = nc.dram_tensor("wq", [P, 16 * G * P], STAT, kind="ExternalInput")
    wk = nc.dram_tensor("wk", [P, 16 * P], STAT, kind="ExternalInput")
    wv = nc.dram_tensor("wv", [P, 16 * P], STAT, kind="ExternalInput")
    wo = nc.dram_tensor("wo", [P, G * C], F16, kind="ExternalInput")
    kpT = nc.dram_tensor("kpT", [P, B * 8 * P], STAT, kind="ExternalInput")
    vpn = nc.dram_tensor("vpn", [P, B * 8 * P], STAT, kind="ExternalInput")
    cosq = nc.dram_tensor("cosq", [P, T], F16, kind="ExternalInput")
    sinq = nc.dram_tensor("sinq", [P, T], F16, kind="ExternalInput")
    cosk = nc.dram_tensor("cosk", [P, T], F16, kind="ExternalInput")
    sink = nc.dram_tensor("sink", [P, T], F16, kind="ExternalInput")
    tri = nc.dram_tensor("tri", [P, P], F16, kind="ExternalInput")
    rperm = nc.dram_tensor("rperm", [P, P], STAT, kind="ExternalInput")
    ones = nc.dram_tensor("ones", [P, P], STAT, kind="ExternalInput")
    ident = nc.dram_tensor("ident", [P, P], STAT, kind="ExternalInput")
    y = nc.dram_tensor("y", [NB, C], F16, kind="ExternalOutput")
    io = (xT, wq, wk, wv, wo, kpT, vpn, cosq, sinq, cosk, sink, tri,
          rperm, ones, ident, y)
    with nc.allow_low_precision(reason="fp16 operands; fp32 accumulation"):
        with tile.TileContext(nc) as tc:
            _emit(tc, io)
    nc.compile()
    return nc


def host_inputs(x, Wq, Wkv, Wo, K_pool, V_pool, slot_map, past_len):
    x = np.asarray(x, dtype=np.float32)
    Wq = np.asarray(Wq, dtype=np.float32)
    Wkv = np.asarray(Wkv, dtype=np.float32)
    Wo = np.asarray(Wo, dtype=np.float32)
    K_pool = np.asarray(K_pool, dtype=np.float32)
    V_pool = np.asarray(V_pool, dtype=np.float32)
    slot_map = np.asarray(slot_map, dtype=np.int32)
    past = int(past_len)
    assert past == PAST, f"kernel hardcodes past_len={PAST}, got {past}"

    xT = np.ascontiguousarray(x.reshape(NB, C).T.astype(np.float16))

    # rope tables; argument arithmetic mirrors the f32 ops of the reference
    idx = np.arange(D // 2, dtype=np.float32)
    inv = np.float32(1.0) / np.float32(10000.0) ** (idx / np.float32(D // 2))
    inv = inv.astype(np.float32)
    t = np.arange(past, past + T, dtype=np.float32)
    freqs = (t[:, None] * inv[None, :]).astype(np.float32)
    emb = np.concatenate([freqs, freqs], axis=1)
    cos = np.cos(emb).astype(np.float32)
    sin = np.sin(emb).astype(np.float32)
    qscale = np.float32(1.0) / np.sqrt(np.float32(D))
    cosqT = np.ascontiguousarray((cos * qscale).T.astype(np.float16))
    sinqT = np.ascontiguousarray((sin * qscale).T.astype(np.float16))
    coskT = np.ascontiguousarray(cos.T.astype(np.float16))
    sinkT = np.ascontiguousarray(sin.T.astype(np.float16))

    # shared [128,128] triangular mask for block-aligned causal diagonals
    s_i = np.arange(P)[:, None]
    u_i = np.arange(P)[None, :]
    tri = np.where(s_i <= u_i, 0.0, NEG).astype(np.float16)

    rperm = np.zeros((P, P), np.float32)
    for d in range(D // 2):
        rperm[d + D // 2, d] = -1.0       # rot(q)[d] = -q[d+64] for d < 64
        rperm[d, d + D // 2] = 1.0        # rot(q)[d] = q[d-64] for d >= 64
    rperm = rperm.astype(NP_BF16)
    ones = np.ones((P, P), NP_BF16)
    ident = np.eye(P, dtype=np.float32).astype(NP_BF16)

    # host-side past-KV gather (+ transpose for K): logical past order
    gs = np.asarray(slot_map[:, :past], dtype=np.int64)     # [B, 1024]
    in_maps = []
    for c in range(NCORES):
        Kg = K_pool[gs, c, :].astype(NP_BF16)               # [B, 1024, 128]
        Vg = V_pool[gs, c, :].astype(NP_BF16)
        # kT_past [d, b, j, s%128]  -> flat [128, B*8*128]
        kpT = np.ascontiguousarray(
            Kg.reshape(B, 8, P, D).transpose(3, 0, 1, 2).reshape(P, B * 8 * P))
        # vg [s%128, b, j, d] -> flat [128, B*8*128]
        vpn = np.ascontiguousarray(
            Vg.reshape(B, 8, P, D).transpose(2, 0, 1, 3).reshape(P, B * 8 * P))
        # weight tiles pre-arranged to [partition, kc*m] so device loads are
        # one contiguous run per partition
        wq_l = Wq[G * D * c:G * D * (c + 1), :].T.reshape(16, P, G * D)
        wq_l = wq_l.transpose(1, 0, 2).reshape(P, 16 * G * D)
        wk_l = Wkv[D * c:D * (c + 1), :].T.reshape(16, P, D)
        wk_l = wk_l.transpose(1, 0, 2).reshape(P, 16 * D)
        wv_l = Wkv[HKV * D + D * c:HKV * D + D * (c + 1), :].T.reshape(16, P, D)
        wv_l = wv_l.transpose(1, 0, 2).reshape(P, 16 * D)
        wo_l = Wo[:, G * D * c:G * D * (c + 1)].T.reshape(G, P, C)
        wo_l = wo_l.transpose(1, 0, 2).reshape(P, G * C)
        in_maps.append({
            "xT": xT,
            "wq": np.ascontiguousarray(wq_l.astype(NP_BF16)),
            "wk": np.ascontiguousarray(wk_l.astype(NP_BF16)),
            "wv": np.ascontiguousarray(wv_l.astype(NP_BF16)),
            "wo": np.ascontiguousarray(wo_l.astype(np.float16)),
            "kpT": kpT, "vpn": vpn,
            "cosq": cosqT, "sinq": sinqT, "cosk": coskT, "sink": sinkT,
            "tri": tri, "rperm": rperm, "ones": ones, "ident": ident,
        })
    return in_maps


_NC_CACHE = None


def kernel(**inputs):
    global _NC_CACHE
    in_maps = host_inputs(**inputs)
    if _NC_CACHE is None:
        _NC_CACHE = build_nc()
    res = run_bass_kernel_spmd(_NC_CACHE, in_maps, core_ids=list(range(NCORES)))
    y = res.results[0]["y"].astype(np.float32)
    for c in range(1, NCORES):
        y = y + res.results[c]["y"].astype(np.float32)
    return y.reshape(B, T, C)


# revision 29
# speedup vs baseline: 1.7300x; 1.0019x over previous
"""Trainium2 Bass kernel for paged causal self-attention (GQA + YaRN rope).

Sharding: tensor-parallel over heads. Core c (of 8) owns kv-head c and
q-heads 2c, 2c+1 for both batches. Each core computes a partial output
y_c = attn_c @ Wo_c.T over its 256 channels; the host sums the 8 partials.

The reference's scatter of new K/V into the pools is dead code w.r.t. the
returned output; new K/V are consumed directly from SBUF. The past-KV
gather (slot_map indexed) and the [s,d]/[d,s] layout transposes are done
on the host, so the device sees two contiguous fp16 layouts.

All matmul operands are fp16 (host-precast); accumulation stays fp32 in
PSUM. exp uses a constant -4 bias (cancels in softmax) for fp16 range
margin. Causal diagonal chunks are column-sliced and share one [128,128]
triangular mask tile.
"""

import sys

sys.path.insert(0, "/opt/trn_rl_repo")

import ml_dtypes
import numpy as np

NP_BF16 = np.dtype(ml_dtypes.bfloat16)

import concourse.bacc as bacc
import concourse.bass as bass
import concourse.tile as tile
from concourse import mybir
from concourse.bass_utils import run_bass_kernel_spmd

F32 = mybir.dt.float32
F16 = mybir.dt.float16
BF16 = mybir.dt.bfloat16
# stationary matmul operands use bf16 (enables fast weight load);
# moving operands stay fp16 for precision
STAT = BF16
EXP = mybir.ActivationFunctionType.Exp

B, T, PAST = 2, 1024, 1024
H, HKV, D = 16, 8, 128
G = H // HKV            # q heads per kv head
C = H * D               # 2048
TOTAL = PAST + T        # 2048
NB = B * T              # 2048 flattened tokens
NCORES = 8
P = 128
TB = 512                # token block
NEG = -60000.0          # mask value (fp16-representable; exp underflows to 0)
EBIAS = -4.0            # constant exp bias; cancels in softmax


def _emit(tc, io):
    nc = tc.nc
    (xT, wq, wk, wv, wo, kpT, vpn, cosq, sinq, cosk, sink, tri, rperm,
     ones, ident, y) = io

    with (
        tc.tile_pool(name="const", bufs=1) as cp,
        tc.tile_pool(name="persist", bufs=1) as pp,
        tc.tile_pool(name="ysb", bufs=4) as yp,
    ):
        # ---- weight tiles (loaded chunked, interleaved with the first x
        # tiles, inside the phase-1 loop) ----
        wq_t = pp.tile([P, 16, G * P], STAT)
        wk_t = pp.tile([P, 16, P], STAT)
        wv_t = pp.tile([P, 16, P], STAT)

        ebias = cp.tile([P, 1], F32)
        nc.vector.memset(ebias[:], EBIAS)

        # ---- remaining constants: SWDGE on the idle GpSimd queue, dep-
        # delayed behind the last weight chunk so they don't steal DMA
        # bandwidth from the startup-critical loads ----
        gdep = cp.tile([P, 8], STAT)
        nc.gpsimd.tensor_copy(gdep[:], wq_t[:, 15, 0:8])
        cosq_t = pp.tile([P, T], F16)
        nc.gpsimd.dma_start(cosq_t[:], cosq[:])
        sinq_t = pp.tile([P, T], F16)
        nc.gpsimd.dma_start(sinq_t[:], sinq[:])
        cosk_t = pp.tile([P, T], F16)
        nc.gpsimd.dma_start(cosk_t[:], cosk[:])
        sink_t = pp.tile([P, T], F16)
        nc.gpsimd.dma_start(sink_t[:], sink[:])
        tri_t = cp.tile([P, P], F16)
        nc.gpsimd.dma_start(tri_t[:], tri[:])
        rperm_t = cp.tile([P, P], STAT)
        nc.gpsimd.dma_start(rperm_t[:], rperm[:])
        ones_t = cp.tile([P, P], STAT)
        nc.gpsimd.dma_start(ones_t[:], ones[:])
        ident_t = cp.tile([P, P], STAT)
        nc.gpsimd.dma_start(ident_t[:], ident[:])
        kT_past = pp.tile([P, B, 8, P], STAT)  # [d, b, chunk, s%128]
        nc.gpsimd.dma_start(kT_past[:], kpT.rearrange("p (b j m) -> p b j m", b=B, j=8))
        vg = pp.tile([P, B, 8, P], STAT)       # [s%128, b, chunk, d]
        nc.gpsimd.dma_start(vg[:], vpn.rearrange("p (b j m) -> p b j m", b=B, j=8))
        wo_t = pp.tile([P, G, C], F16)
        nc.gpsimd.dma_start(wo_t[:], wo.rearrange("p (g m) -> p g m", g=G))

        # ---- persistent activations ----
        qT0 = pp.tile([P, NB], F16)       # q head 2c,   [d, token]  (moving)
        qT1 = pp.tile([P, NB], F16)       # q head 2c+1
        kT_new = pp.tile([P, NB], STAT)   # new keys,    [d, token]  (stationary)
        v_nat = pp.tile([P, B, 8, P], STAT)    # new values, [t%128, b, chunk, d]
        att0 = pp.tile([P, NB], STAT)     # attention out head 2c, [d, token]
        att1 = pp.tile([P, NB], STAT)

        # ================= phase 1: projections + rope =================
        with (
            tc.tile_pool(name="xt", bufs=6) as xp,
            tc.tile_pool(name="rope", bufs=3) as rp,
            tc.tile_pool(name="pproj", bufs=1, space="PSUM") as pjp,
            tc.tile_pool(name="prope", bufs=2, space="PSUM") as rpp,
            tc.tile_pool(name="ptr", bufs=2, space="PSUM") as trp,
        ):
            # Rope/v-transpose emission is deferred one block: the PE's
            # in-order stream gets block N+1's projection matmuls BEFORE
            # block N's rope matmuls, so PSUM-evacuation copies (ACT/DVE)
            # have a full block of slack to land.
            pending = None

            def emit_rope(state):
                tbp, raw_k_, raw_q0_, raw_q1_, vsb_ = state
                n0p = tbp * TB
                bp = tbp // 2
                tposp = (tbp % 2) * TB
                # v transposes first (their input was evacuated earliest);
                # all four land in one PSUM tile, one batched evacuation
                vtp = trp.tile([P, 4, P], STAT, name="vtp", tag="vtp")
                for j4 in range(TB // P):
                    nc.tensor.transpose(vtp[:, j4, :],
                                        vsb_[:, j4 * P:(j4 + 1) * P], ident_t[:])
                for raw, dst, ct, stt in (
                    (raw_k_, kT_new, cosk_t, sink_t),
                    (raw_q0_, qT0, cosq_t, sinq_t),
                    (raw_q1_, qT1, cosq_t, sinq_t),
                ):
                    rot = rpp.tile([P, TB], F32, name="rot", tag="rot")
                    nc.tensor.matmul(rot[:], rperm_t[:], raw[:], start=True,
                                     stop=True)
                    t1 = rp.tile([P, TB], F16, name="t1", tag="t1")
                    nc.vector.tensor_mul(t1[:], raw[:], ct[:, tposp:tposp + TB])
                    t2 = rp.tile([P, TB], F16, name="t2", tag="t2")
                    nc.vector.tensor_mul(t2[:], rot[:], stt[:, tposp:tposp + TB])
                    nc.vector.tensor_add(dst[:, n0p:n0p + TB], t1[:], t2[:])
                nc.vector.tensor_copy(
                    v_nat[:, bp, (tbp % 2) * 4:(tbp % 2) * 4 + 4, :], vtp[:])

            # block order (2,3,0,1): the last-roped block (1) is only needed
            # by the final attention groups, so phase 2 starts without
            # waiting for the phase-1 tail.
            for ti, tb in enumerate((2, 3, 0, 1)):
                n0 = tb * TB

                # previous block's q psum evacuations (ACT queue, ready now)
                if pending is not None:
                    tbp, q0p_, q1p_, raw_k_, vsb_ = pending
                    raw_q0 = rp.tile([P, TB], F16, name="raw_q0", tag="rawq0")
                    nc.scalar.copy(raw_q0[:], q0p_[:])
                    raw_q1 = rp.tile([P, TB], F16, name="raw_q1", tag="rawq1")
                    nc.scalar.copy(raw_q1[:], q1p_[:])
                    pending = (tbp, raw_k_, raw_q0, raw_q1, vsb_)

                q0p = pjp.tile([P, TB], F32, name="q0p", tag="q0")
                q1p = pjp.tile([P, TB], F32, name="q1p", tag="q1")
                kkp = pjp.tile([P, TB], F32, name="kkp", tag="kk")
                vvp = pjp.tile([P, TB], F32, name="vvp", tag="vv")
                for kcg in range(4):
                    if ti == 0:
                        # first block: interleave the weight-chunk loads with
                        # the x stream so the first matmul starts early
                        nc.sync.dma_start(
                            wq_t[:, 4 * kcg:4 * kcg + 4, :],
                            wq[:, kcg * 4 * G * P:(kcg + 1) * 4 * G * P]
                            .rearrange("p (kc m) -> p kc m", kc=4))
                        nc.sync.dma_start(
                            wk_t[:, 4 * kcg:4 * kcg + 4, :],
                            wk[:, kcg * 4 * P:(kcg + 1) * 4 * P]
                            .rearrange("p (kc m) -> p kc m", kc=4))
                        nc.sync.dma_start(
                            wv_t[:, 4 * kcg:4 * kcg + 4, :],
                            wv[:, kcg * 4 * P:(kcg + 1) * 4 * P]
                            .rearrange("p (kc m) -> p kc m", kc=4))
                    xt4 = xp.tile([P, 4, TB], F16, name="xt4", tag="xt")
                    nc.sync.dma_start(xt4[:],
                                      xT[:, 4 * kcg:4 * kcg + 4, n0:n0 + TB])
                    for kc4 in range(4):
                        kc = 4 * kcg + kc4
                        xt = xt4[:, kc4, :]
                        st = (kc == 0)
                        sp = (kc == 15)
                        nc.tensor.matmul(q0p[:], wq_t[:, kc, 0:P], xt, start=st, stop=sp)
                        nc.tensor.matmul(q1p[:], wq_t[:, kc, P:2 * P], xt, start=st, stop=sp)
                        nc.tensor.matmul(kkp[:], wk_t[:, kc, :], xt, start=st, stop=sp)
                        nc.tensor.matmul(vvp[:], wv_t[:, kc, :], xt, start=st, stop=sp)

                # evacuate kk/vv immediately on separate engines
                raw_k = rp.tile([P, TB], F16, name="raw_k", tag="rawk")
                nc.scalar.copy(raw_k[:], kkp[:])
                vsb = rp.tile([P, TB], STAT, name="vsb", tag="vsb")
                nc.vector.tensor_copy(vsb[:], vvp[:])

                if pending is not None:
                    emit_rope(pending)
                pending = (tb, q0p, q1p, raw_k, vsb)

            # drain the final block
            tbp, q0p_, q1p_, raw_k_, vsb_ = pending
            raw_q0 = rp.tile([P, TB], F16, name="raw_q0", tag="rawq0")
            nc.scalar.copy(raw_q0[:], q0p_[:])
            raw_q1 = rp.tile([P, TB], F16, name="raw_q1", tag="rawq1")
            nc.scalar.copy(raw_q1[:], q1p_[:])
            emit_rope((tbp, raw_k_, raw_q0, raw_q1, vsb_))

        # ================= phase 2+3: attention + output proj =================
        with (
            tc.tile_pool(name="exps", bufs=2) as ep,
            tc.tile_pool(name="sums", bufs=2) as sp_,
            tc.tile_pool(name="attw", bufs=2) as aw,
            tc.tile_pool(name="pscore", bufs=2, space="PSUM") as scp,
            tc.tile_pool(name="pav", bufs=2, space="PSUM") as avp,
            tc.tile_pool(name="pbc", bufs=2, space="PSUM") as bcp,
            tc.tile_pool(name="py", bufs=2, space="PSUM") as pyp,
        ):
            for b in range(B):
                for tbq in range(2):             # query block of 512 in batch
                    t0 = b * T + tbq * TB
                    for g, (qT, att) in enumerate(((qT0, att0), (qT1, att1))):
                        q_ap = qT[:, t0:t0 + TB]
                        njnew = 4 * tbq + 4
                        nch = 8 + njnew

                        # chunk list: past (full), sliced-diag new (desc ri),
                        # then full new; last chunk is full-width.
                        chunks = [(kT_past[:, b, j, :], vg[:, b, j, :], None)
                                  for j in range(8)]
                        sliced = []
                        full_new = []
                        for j in range(njnew):
                            koff = b * T + j * P
                            ri = j - 4 * tbq
                            ent = (kT_new[:, koff:koff + P],
                                   v_nat[:, b, j, :], ri if ri > 0 else None,
                                   ri == 0)
                            if ri > 0:
                                sliced.append(ent)
                            else:
                                full_new.append(ent)
                        sliced.reverse()   # descending ri
                        # order: past, sliced (ri 3..1), full new (ri<0), ri==0 last
                        full_new.sort(key=lambda e: e[3])
                        chunks = ([(k_, v_, None, False) for k_, v_, _ in chunks]
                                  + sliced + full_new)

                        expS = ep.tile([P, 16, TB], F16, name="expS", tag="expS")
                        sumP = sp_.tile([P, TB], F16, name="sumP", tag="sumP")
                        av = avp.tile([P, TB], F32, name="av", tag="av")

                        # scores run one chunk ahead of av on the PE so the
                        # exp (ACT) latency of chunk ci hides under the
                        # score matmul of chunk ci+1
                        pend_av = None
                        for ci, (k_ap, v_ap, ri, diag0) in enumerate(chunks):
                            c0 = 0 if ri is None else P * ri
                            s_ps = scp.tile([P, TB], F32, name="s_ps", tag="s")
                            nc.tensor.matmul(s_ps[:, c0:], k_ap, q_ap[:, c0:],
                                             start=True, stop=True)
                            if ri is not None or diag0:
                                nc.vector.tensor_add(s_ps[:, c0:c0 + P],
                                                     s_ps[:, c0:c0 + P], tri_t[:])
                            e_ap = expS[:, ci, c0:]
                            nc.scalar.activation(e_ap, s_ps[:, c0:], EXP,
                                                 bias=ebias[:])
                            if ci == 0:
                                nc.vector.tensor_copy(sumP[:], e_ap)
                            else:
                                nc.vector.tensor_add(sumP[:, c0:], sumP[:, c0:],
                                                     e_ap)
                            if pend_av is not None:
                                pv_ap, pe_ap, pc0, pci = pend_av
                                nc.tensor.matmul(av[:, pc0:], pv_ap, pe_ap,
                                                 start=(pci == 0), stop=False)
                            pend_av = (v_ap, e_ap, c0, ci)
                        pv_ap, pe_ap, pc0, pci = pend_av
                        nc.tensor.matmul(av[:, pc0:], pv_ap, pe_ap,
                                         start=False, stop=True)

                        # softmax denominator: partition-reduce + broadcast via
                        # ones matmul, then fast approx reciprocal
                        rbc = bcp.tile([P, TB], F32, name="rbc", tag="rbc")
                        nc.tensor.matmul(rbc[:], ones_t[:], sumP[:],
                                         start=True, stop=True)
                        rinv = aw.tile([P, TB], F32, name="rinv", tag="rinv")
                        nc.vector.reciprocal_approx_fast(rinv[:], rbc[:])
                        nc.vector.tensor_mul(att[:, t0:t0 + TB], av[:], rinv[:])

                    # output projection for these 512 tokens; evacuate the
                    # four 512-ch chunks into one wide tile, store with a
                    # single contiguous DMA per 128 tokens
                    for tc4 in range(4):
                        tt0 = t0 + tc4 * P
                        ysb = yp.tile([P, 4, TB], F16, name="ysbt", tag="ysbt")
                        for cb in range(4):
                            yps = pyp.tile([P, TB], F32, name="yps", tag="y")
                            nc.tensor.matmul(yps[:], att0[:, tt0:tt0 + P],
                                             wo_t[:, 0, cb * TB:(cb + 1) * TB],
                                             start=True, stop=False)
                            nc.tensor.matmul(yps[:], att1[:, tt0:tt0 + P],
                                             wo_t[:, 1, cb * TB:(cb + 1) * TB],
                                             start=False, stop=True)
                            if (tc4 + cb) % 2 == 0:
                                nc.scalar.copy(ysb[:, cb, :], yps[:])
                            else:
                                nc.vector.tensor_copy(ysb[:, cb, :], yps[:])
                        nc.sync.dma_start(y[tt0:tt0 + P, :], ysb[:])


def build_nc():
    nc = bacc.Bacc("TRN2")
    xT = nc.dram_tensor("xT", [P, 16, NB], F16, kind="ExternalInput")
    wq = nc.dram_tensor("wq", [P, 16 * G * P], STAT, kind="ExternalInput")
    wk = nc.dram_tensor("wk", [P, 16 * P], STAT, kind="ExternalInput")
    wv = nc.dram_tensor("wv", [P, 16 * P], STAT, kind="ExternalInput")
    wo = nc.dram_tensor("wo", [P, G * C], F16, kind="ExternalInput")
    kpT = nc.dram_tensor("kpT", [P, B * 8 * P], STAT, kind="ExternalInput")
    vpn = nc.dram_tensor("vpn", [P, B * 8 * P], STAT, kind="ExternalInput")
    cosq = nc.dram_tensor("cosq", [P, T], F16, kind="ExternalInput")
    sinq = nc.dram_tensor("sinq", [P, T], F16, kind="ExternalInput")
    cosk = nc.dram_tensor("cosk", [P, T], F16, kind="ExternalInput")
    sink = nc.dram_tensor("sink", [P, T], F16, kind="ExternalInput")
    tri = nc.dram_tensor("tri", [P, P], F16, kind="ExternalInput")
    rperm = nc.dram_tensor("rperm", [P, P], STAT, kind="ExternalInput")
    ones = nc.dram_tensor("ones", [P, P], STAT, kind="ExternalInput")
    ident = nc.dram_tensor("ident", [P, P], STAT, kind="ExternalInput")
    y = nc.dram_tensor("y", [NB, C], F16, kind="ExternalOutput")
    io = (xT, wq, wk, wv, wo, kpT, vpn, cosq, sinq, cosk, sink, tri,
          rperm, ones, ident, y)
    with nc.allow_low_precision(reason="fp16 operands; fp32 accumulation"):
        with tile.TileContext(nc) as tc:
            _emit(tc, io)
    nc.compile()
    return nc


def host_inputs(x, Wq, Wkv, Wo, K_pool, V_pool, slot_map, past_len):
    x = np.asarray(x, dtype=np.float32)
    Wq = np.asarray(Wq, dtype=np.float32)
    Wkv = np.asarray(Wkv, dtype=np.float32)
    Wo = np.asarray(Wo, dtype=np.float32)
    K_pool = np.asarray(K_pool, dtype=np.float32)
    V_pool = np.asarray(V_pool, dtype=np.float32)
    slot_map = np.asarray(slot_map, dtype=np.int32)
    past = int(past_len)
    assert past == PAST, f"kernel hardcodes past_len={PAST}, got {past}"

    # [p, kc, tok]: per-partition contiguous x tiles
    xT = np.ascontiguousarray(
        x.reshape(NB, C).T.reshape(16, P, NB).transpose(1, 0, 2)
        .astype(np.float16))

    # rope tables; argument arithmetic mirrors the f32 ops of the reference
    idx = np.arange(D // 2, dtype=np.float32)
    inv = np.float32(1.0) / np.float32(10000.0) ** (idx / np.float32(D // 2))
    inv = inv.astype(np.float32)
    t = np.arange(past, past + T, dtype=np.float32)
    freqs = (t[:, None] * inv[None, :]).astype(np.float32)
    emb = np.concatenate([freqs, freqs], axis=1)
    cos = np.cos(emb).astype(np.float32)
    sin = np.sin(emb).astype(np.float32)
    qscale = np.float32(1.0) / np.sqrt(np.float32(D))
    cosqT = np.ascontiguousarray((cos * qscale).T.astype(np.float16))
    sinqT = np.ascontiguousarray((sin * qscale).T.astype(np.float16))
    coskT = np.ascontiguousarray(cos.T.astype(np.float16))
    sinkT = np.ascontiguousarray(sin.T.astype(np.float16))

    # shared [128,128] triangular mask for block-aligned causal diagonals
    s_i = np.arange(P)[:, None]
    u_i = np.arange(P)[None, :]
    tri = np.where(s_i <= u_i, 0.0, NEG).astype(np.float16)

    rperm = np.zeros((P, P), np.float32)
    for d in range(D // 2):
        rperm[d + D // 2, d] = -1.0       # rot(q)[d] = -q[d+64] for d < 64
        rperm[d, d + D // 2] = 1.0        # rot(q)[d] = q[d-64] for d >= 64
    rperm = rperm.astype(NP_BF16)
    ones = np.ones((P, P), NP_BF16)
    ident = np.eye(P, dtype=np.float32).astype(NP_BF16)

    # host-side past-KV gather (+ transpose for K): logical past order
    gs = np.asarray(slot_map[:, :past], dtype=np.int64)     # [B, 1024]
    in_maps = []
    for c in range(NCORES):
        Kg = K_pool[gs, c, :].astype(NP_BF16)               # [B, 1024, 128]
        Vg = V_pool[gs, c, :].astype(NP_BF16)
        # kT_past [d, b, j, s%128]  -> flat [128, B*8*128]
        kpT = np.ascontiguousarray(
            Kg.reshape(B, 8, P, D).transpose(3, 0, 1, 2).reshape(P, B * 8 * P))
        # vg [s%128, b, j, d] -> flat [128, B*8*128]
        vpn = np.ascontiguousarray(
            Vg.reshape(B, 8, P, D).transpose(2, 0, 1, 3).reshape(P, B * 8 * P))
        # weight tiles pre-arranged to [partition, kc*m] so device loads are
        # one contiguous run per partition
        wq_l = Wq[G * D * c:G * D * (c + 1), :].T.reshape(16, P, G * D)
        wq_l = wq_l.transpose(1, 0, 2).reshape(P, 16 * G * D)
        wk_l = Wkv[D * c:D * (c + 1), :].T.reshape(16, P, D)
        wk_l = wk_l.transpose(1, 0, 2).reshape(P, 16 * D)
        wv_l = Wkv[HKV * D + D * c:HKV * D + D * (c + 1), :].T.reshape(16, P, D)
        wv_l = wv_l.transpose(1, 0, 2).reshape(P, 16 * D)
        wo_l = Wo[:, G * D * c:G * D * (c + 1)].T.reshape(G, P, C)
        wo_l = wo_l.transpose(1, 0, 2).reshape(P, G * C)
        in_maps.append({
            "xT": xT,
            "wq": np.ascontiguousarray(wq_l.astype(NP_BF16)),
            "wk": np.ascontiguousarray(wk_l.astype(NP_BF16)),
            "wv": np.ascontiguousarray(wv_l.astype(NP_BF16)),
            "wo": np.ascontiguousarray(wo_l.astype(np.float16)),
            "kpT": kpT, "vpn": vpn,
            "cosq": cosqT, "sinq": sinqT, "cosk": coskT, "sink": sinkT,
            "tri": tri, "rperm": rperm, "ones": ones, "ident": ident,
        })
    return in_maps


_NC_CACHE = None


def kernel(**inputs):
    global _NC_CACHE
    in_maps = host_inputs(**inputs)
    if _NC_CACHE is None:
        _NC_CACHE = build_nc()
    res = run_bass_kernel_spmd(_NC_CACHE, in_maps, core_ids=list(range(NCORES)))
    y = res.results[0]["y"].astype(np.float32)
    for c in range(1, NCORES):
        y = y + res.results[c]["y"].astype(np.float32)
    return y.reshape(B, T, C)


# revision 34
# speedup vs baseline: 1.8155x; 1.0494x over previous
"""Trainium2 Bass kernel for paged causal self-attention (GQA + YaRN rope).

Sharding: tensor-parallel over heads. Core c (of 8) owns kv-head c and
q-heads 2c, 2c+1 for both batches. Each core computes a partial output
y_c = attn_c @ Wo_c.T over its 256 channels; the host sums the 8 partials.

The reference's scatter of new K/V into the pools is dead code w.r.t. the
returned output; new K/V are consumed directly from SBUF. The past-KV
gather (slot_map indexed) and the [s,d]/[d,s] layout transposes are done
on the host, so the device sees two contiguous fp16 layouts.

All matmul operands are fp16 (host-precast); accumulation stays fp32 in
PSUM. exp uses a constant -4 bias (cancels in softmax) for fp16 range
margin. Causal diagonal chunks are column-sliced and share one [128,128]
triangular mask tile.
"""

import sys

sys.path.insert(0, "/opt/trn_rl_repo")

import ml_dtypes
import numpy as np

NP_BF16 = np.dtype(ml_dtypes.bfloat16)

import concourse.bacc as bacc
import concourse.bass as bass
import concourse.tile as tile
from concourse import mybir
from concourse.bass_utils import run_bass_kernel_spmd

F32 = mybir.dt.float32
F16 = mybir.dt.float16
BF16 = mybir.dt.bfloat16
# stationary matmul operands use bf16 (enables fast weight load);
# moving operands stay fp16 for precision
STAT = BF16
EXP = mybir.ActivationFunctionType.Exp

B, T, PAST = 2, 1024, 1024
H, HKV, D = 16, 8, 128
G = H // HKV            # q heads per kv head
C = H * D               # 2048
TOTAL = PAST + T        # 2048
NB = B * T              # 2048 flattened tokens
NCORES = 8
P = 128
TB = 512                # token block
NEG = -60000.0          # mask value (fp16-representable; exp underflows to 0)
EBIAS = -4.0            # constant exp bias; cancels in softmax


def _emit(tc, io):
    nc = tc.nc
    (xT, wq, wk, wv, wo, kpT, vpn, cosq, sinq, cosk, sink, tri, rperm,
     ones, ident, y) = io

    with (
        tc.tile_pool(name="const", bufs=1) as cp,
        tc.tile_pool(name="persist", bufs=1) as pp,
        tc.tile_pool(name="ysb", bufs=4) as yp,
    ):
        # ---- weight tiles (loaded chunked, interleaved with the first x
        # tiles, inside the phase-1 loop) ----
        wq_t = pp.tile([P, 16, G * P], STAT)
        wk_t = pp.tile([P, 16, P], STAT)
        wv_t = pp.tile([P, 16, P], STAT)

        ebias = cp.tile([P, 1], F32)
        nc.vector.memset(ebias[:], EBIAS)

        # ---- remaining constant tiles; their loads are emitted inside the
        # first phase-1 block (SWDGE on the idle GpSimd queue, dep-delayed
        # behind the last weight chunk so they don't steal DMA bandwidth
        # from the startup-critical loads) ----
        gdep = cp.tile([P, 8], STAT)
        cosq_t = pp.tile([P, T], F16)
        sinq_t = pp.tile([P, T], F16)
        cosk_t = pp.tile([P, T], F16)
        sink_t = pp.tile([P, T], F16)
        tri_t = cp.tile([P, P], F16)
        rperm_t = cp.tile([P, P], STAT)
        ones_t = cp.tile([P, P], STAT)
        ident_t = cp.tile([P, P], STAT)
        kT_past = pp.tile([P, B, 8, P], STAT)  # [d, b, chunk, s%128]
        vg = pp.tile([P, B, 8, P], STAT)       # [s%128, b, chunk, d]
        wo_t = pp.tile([P, G, C], F16)

        def emit_const_loads():
            nc.gpsimd.tensor_copy(gdep[:], wq_t[:, 15, 0:8])
            nc.gpsimd.dma_start(cosq_t[:], cosq[:])
            nc.gpsimd.dma_start(sinq_t[:], sinq[:])
            nc.gpsimd.dma_start(cosk_t[:], cosk[:])
            nc.gpsimd.dma_start(sink_t[:], sink[:])
            nc.gpsimd.dma_start(tri_t[:], tri[:])
            nc.gpsimd.dma_start(rperm_t[:], rperm[:])
            nc.gpsimd.dma_start(ones_t[:], ones[:])
            nc.gpsimd.dma_start(ident_t[:], ident[:])
            nc.gpsimd.dma_start(kT_past[:],
                                kpT.rearrange("p (b j m) -> p b j m", b=B, j=8))
            nc.gpsimd.dma_start(vg[:],
                                vpn.rearrange("p (b j m) -> p b j m", b=B, j=8))
            nc.gpsimd.dma_start(wo_t[:], wo.rearrange("p (g m) -> p g m", g=G))

        # ---- persistent activations ----
        qT0 = pp.tile([P, NB], F16)       # q head 2c,   [d, token]  (moving)
        qT1 = pp.tile([P, NB], F16)       # q head 2c+1
        kT_new = pp.tile([P, NB], STAT)   # new keys,    [d, token]  (stationary)
        v_nat = pp.tile([P, B, 8, P], STAT)    # new values, [t%128, b, chunk, d]
        att0 = pp.tile([P, NB], STAT)     # attention out head 2c, [d, token]
        att1 = pp.tile([P, NB], STAT)

        # ================= phase 1: projections + rope =================
        with (
            tc.tile_pool(name="xt", bufs=6) as xp,
            tc.tile_pool(name="rope", bufs=3) as rp,
            tc.tile_pool(name="pproj", bufs=1, space="PSUM") as pjp,
            tc.tile_pool(name="prope", bufs=2, space="PSUM") as rpp,
            tc.tile_pool(name="ptr", bufs=2, space="PSUM") as trp,
        ):
            # Rope/v-transpose emission is deferred one block: the PE's
            # in-order stream gets block N+1's projection matmuls BEFORE
            # block N's rope matmuls, so PSUM-evacuation copies (ACT/DVE)
            # have a full block of slack to land.
            pending = None

            def emit_rope(state):
                tbp, raw_k_, raw_q0_, raw_q1_, vsb_ = state
                n0p = tbp * TB
                bp = tbp // 2
                tposp = (tbp % 2) * TB
                # v transposes first (their input was evacuated earliest);
                # all four land in one PSUM tile, one batched evacuation
                vtp = trp.tile([P, 4, P], STAT, name="vtp", tag="vtp")
                for j4 in range(TB // P):
                    nc.tensor.transpose(vtp[:, j4, :],
                                        vsb_[:, j4 * P:(j4 + 1) * P], ident_t[:])
                for raw, dst, ct, stt in (
                    (raw_k_, kT_new, cosk_t, sink_t),
                    (raw_q0_, qT0, cosq_t, sinq_t),
                    (raw_q1_, qT1, cosq_t, sinq_t),
                ):
                    rot = rpp.tile([P, TB], F32, name="rot", tag="rot")
                    nc.tensor.matmul(rot[:], rperm_t[:], raw[:], start=True,
                                     stop=True)
                    t1 = rp.tile([P, TB], F16, name="t1", tag="t1")
                    nc.vector.tensor_mul(t1[:], raw[:], ct[:, tposp:tposp + TB])
                    t2 = rp.tile([P, TB], F16, name="t2", tag="t2")
                    nc.vector.tensor_mul(t2[:], rot[:], stt[:, tposp:tposp + TB])
                    nc.vector.tensor_add(dst[:, n0p:n0p + TB], t1[:], t2[:])
                nc.vector.tensor_copy(
                    v_nat[:, bp, (tbp % 2) * 4:(tbp % 2) * 4 + 4, :], vtp[:])

            # block order (2,3,0,1): the last-roped block (1) is only needed
            # by the final attention groups, so phase 2 starts without
            # waiting for the phase-1 tail.
            for ti, tb in enumerate((2, 3, 0, 1)):
                n0 = tb * TB

                # previous block's q psum evacuations (ACT queue, ready now)
                if pending is not None:
                    tbp, q0p_, q1p_, raw_k_, vsb_ = pending
                    raw_q0 = rp.tile([P, TB], F16, name="raw_q0", tag="rawq0")
                    nc.scalar.copy(raw_q0[:], q0p_[:])
                    raw_q1 = rp.tile([P, TB], F16, name="raw_q1", tag="rawq1")
                    nc.scalar.copy(raw_q1[:], q1p_[:])
                    pending = (tbp, raw_k_, raw_q0, raw_q1, vsb_)

                q0p = pjp.tile([P, TB], F32, name="q0p", tag="q0")
                q1p = pjp.tile([P, TB], F32, name="q1p", tag="q1")
                kkp = pjp.tile([P, TB], F32, name="kkp", tag="kk")
                vvp = pjp.tile([P, TB], F32, name="vvp", tag="vv")
                for kcg in range(4):
                    if ti == 0:
                        # first block: interleave the weight-chunk loads with
                        # the x stream so the first matmul starts early
                        nc.sync.dma_start(
                            wq_t[:, 4 * kcg:4 * kcg + 4, :],
                            wq[:, kcg * 4 * G * P:(kcg + 1) * 4 * G * P]
                            .rearrange("p (kc m) -> p kc m", kc=4))
                        nc.sync.dma_start(
                            wk_t[:, 4 * kcg:4 * kcg + 4, :],
                            wk[:, kcg * 4 * P:(kcg + 1) * 4 * P]
                            .rearrange("p (kc m) -> p kc m", kc=4))
                        nc.sync.dma_start(
                            wv_t[:, 4 * kcg:4 * kcg + 4, :],
                            wv[:, kcg * 4 * P:(kcg + 1) * 4 * P]
                            .rearrange("p (kc m) -> p kc m", kc=4))
                    xt4 = xp.tile([P, 4, TB], F16, name="xt4", tag="xt")
                    nc.sync.dma_start(xt4[:],
                                      xT[:, 4 * kcg:4 * kcg + 4, n0:n0 + TB])
                    for kc4 in range(4):
                        kc = 4 * kcg + kc4
                        xt = xt4[:, kc4, :]
                        st = (kc == 0)
                        sp = (kc == 15)
                        nc.tensor.matmul(q0p[:], wq_t[:, kc, 0:P], xt, start=st, stop=sp)
                        nc.tensor.matmul(q1p[:], wq_t[:, kc, P:2 * P], xt, start=st, stop=sp)
                        nc.tensor.matmul(kkp[:], wk_t[:, kc, :], xt, start=st, stop=sp)
                        nc.tensor.matmul(vvp[:], wv_t[:, kc, :], xt, start=st, stop=sp)

                # evacuate kk/vv immediately on separate engines
                raw_k = rp.tile([P, TB], F16, name="raw_k", tag="rawk")
                nc.scalar.copy(raw_k[:], kkp[:])
                vsb = rp.tile([P, TB], STAT, name="vsb", tag="vsb")
                nc.vector.tensor_copy(vsb[:], vvp[:])

                if ti == 0:
                    emit_const_loads()
                if pending is not None:
                    emit_rope(pending)
                pending = (tb, q0p, q1p, raw_k, vsb)

            # drain the final block (q0/q1 evacuations on separate engines —
            # no later projection matmuls will hide their latency)
            tbp, q0p_, q1p_, raw_k_, vsb_ = pending
            raw_q0 = rp.tile([P, TB], F16, name="raw_q0", tag="rawq0")
            nc.scalar.copy(raw_q0[:], q0p_[:])
            raw_q1 = rp.tile([P, TB], F16, name="raw_q1", tag="rawq1")
            nc.vector.tensor_copy(raw_q1[:], q1p_[:])
            emit_rope((tbp, raw_k_, raw_q0, raw_q1, vsb_))

        # ================= phase 2+3: attention + output proj =================
        with (
            tc.tile_pool(name="exps", bufs=2) as ep,
            tc.tile_pool(name="sums", bufs=2) as sp_,
            tc.tile_pool(name="attw", bufs=2) as aw,
            tc.tile_pool(name="pscore", bufs=3, space="PSUM") as scp,
            tc.tile_pool(name="pav", bufs=2, space="PSUM") as avp,
            tc.tile_pool(name="pbc", bufs=1, space="PSUM") as bcp,
            tc.tile_pool(name="py", bufs=2, space="PSUM") as pyp,
        ):
            def emit_outproj(t0_):
                # output projection for 512 tokens; evacuate the four 512-ch
                # chunks into one wide tile, one contiguous DMA per 128 toks
                for tc4 in range(4):
                    tt0 = t0_ + tc4 * P
                    ysb = yp.tile([P, 4, TB], F16, name="ysbt", tag="ysbt")
                    for cb in range(4):
                        yps = pyp.tile([P, TB], F32, name="yps", tag="y")
                        nc.tensor.matmul(yps[:], att0[:, tt0:tt0 + P],
                                         wo_t[:, 0, cb * TB:(cb + 1) * TB],
                                         start=True, stop=False)
                        nc.tensor.matmul(yps[:], att1[:, tt0:tt0 + P],
                                         wo_t[:, 1, cb * TB:(cb + 1) * TB],
                                         start=False, stop=True)
                        if (tc4 + cb) % 2 == 0:
                            nc.scalar.copy(ysb[:, cb, :], yps[:])
                        else:
                            nc.vector.tensor_copy(ysb[:, cb, :], yps[:])
                    nc.sync.dma_start(y[tt0:tt0 + P, :], ysb[:])

            pend_out = None
            for b in range(B):
                for tbq in range(2):             # query block of 512 in batch
                    t0 = b * T + tbq * TB
                    for g, (qT, att) in enumerate(((qT0, att0), (qT1, att1))):
                        if g == 1 and pend_out is not None:
                            # previous block's output projection, emitted
                            # here so its att inputs have a chunk-loop of
                            # slack and its matmuls fill exp-latency bubbles
                            emit_outproj(pend_out)
                            pend_out = None
                        q_ap = qT[:, t0:t0 + TB]
                        njnew = 4 * tbq + 4
                        nch = 8 + njnew

                        # chunk list: past (full), sliced-diag new (desc ri),
                        # then full new; last chunk is full-width.
                        chunks = [(kT_past[:, b, j, :], vg[:, b, j, :], None)
                                  for j in range(8)]
                        sliced = []
                        full_new = []
                        for j in range(njnew):
                            koff = b * T + j * P
                            ri = j - 4 * tbq
                            ent = (kT_new[:, koff:koff + P],
                                   v_nat[:, b, j, :], ri if ri > 0 else None,
                                   ri == 0)
                            if ri > 0:
                                sliced.append(ent)
                            else:
                                full_new.append(ent)
                        sliced.reverse()   # descending ri
                        # order: past, sliced (ri 3..1), full new (ri<0), ri==0 last
                        full_new.sort(key=lambda e: e[3])
                        chunks = ([(k_, v_, None, False) for k_, v_, _ in chunks]
                                  + sliced + full_new)

                        expS = ep.tile([P, 16, TB], F16, name="expS", tag="expS")
                        sumP = sp_.tile([P, TB], F16, name="sumP", tag="sumP")
                        av = avp.tile([P, TB], F32, name="av", tag="av")

                        # scores run one chunk ahead of av on the PE so the
                        # exp (ACT) latency of chunk ci hides under the
                        # score matmul of chunk ci+1
                        pend_av = None
                        for ci, (k_ap, v_ap, ri, diag0) in enumerate(chunks):
                            c0 = 0 if ri is None else P * ri
                            s_ps = scp.tile([P, TB], F32, name="s_ps", tag="s")
                            nc.tensor.matmul(s_ps[:, c0:], k_ap, q_ap[:, c0:],
                                             start=True, stop=True)
                            if ri is not None or diag0:
                                nc.vector.tensor_add(s_ps[:, c0:c0 + P],
                                                     s_ps[:, c0:c0 + P], tri_t[:])
                            e_ap = expS[:, ci, c0:]
                            nc.scalar.activation(e_ap, s_ps[:, c0:], EXP,
                                                 bias=ebias[:])
                            if ci == 0:
                                nc.vector.tensor_copy(sumP[:], e_ap)
                            else:
                                nc.vector.tensor_add(sumP[:, c0:], sumP[:, c0:],
                                                     e_ap)
                            if pend_av is not None:
                                pv_ap, pe_ap, pc0, pci = pend_av
                                nc.tensor.matmul(av[:, pc0:], pv_ap, pe_ap,
                                                 start=(pci == 0), stop=False)
                            pend_av = (v_ap, e_ap, c0, ci)
                        pv_ap, pe_ap, pc0, pci = pend_av
                        nc.tensor.matmul(av[:, pc0:], pv_ap, pe_ap,
                                         start=False, stop=True)

                        # softmax denominator: partition-reduce + broadcast via
                        # ones matmul, then fast approx reciprocal
                        rbc = bcp.tile([P, TB], F32, name="rbc", tag="rbc")
                        nc.tensor.matmul(rbc[:], ones_t[:], sumP[:],
                                         start=True, stop=True)
                        rinv = aw.tile([P, TB], F32, name="rinv", tag="rinv")
                        nc.vector.reciprocal_approx_fast(rinv[:], rbc[:])
                        nc.vector.tensor_mul(att[:, t0:t0 + TB], av[:], rinv[:])

                    pend_out = t0
            emit_outproj(pend_out)


def build_nc():
    nc = bacc.Bacc("TRN2")
    xT = nc.dram_tensor("xT", [P, 16, NB], F16, kind="ExternalInput")
    wq = nc.dram_tensor("wq", [P, 16 * G * P], STAT, kind="ExternalInput")
    wk = nc.dram_tensor("wk", [P, 16 * P], STAT, kind="ExternalInput")
    wv = nc.dram_tensor("wv", [P, 16 * P], STAT, kind="ExternalInput")
    wo = nc.dram_tensor("wo", [P, G * C], F16, kind="ExternalInput")
    kpT = nc.dram_tensor("kpT", [P, B * 8 * P], STAT, kind="ExternalInput")
    vpn = nc.dram_tensor("vpn", [P, B * 8 * P], STAT, kind="ExternalInput")
    cosq = nc.dram_tensor("cosq", [P, T], F16, kind="ExternalInput")
    sinq = nc.dram_tensor("sinq", [P, T], F16, kind="ExternalInput")
    cosk = nc.dram_tensor("cosk", [P, T], F16, kind="ExternalInput")
    sink = nc.dram_tensor("sink", [P, T], F16, kind="ExternalInput")
    tri = nc.dram_tensor("tri", [P, P], F16, kind="ExternalInput")
    rperm = nc.dram_tensor("rperm", [P, P], STAT, kind="ExternalInput")
    ones = nc.dram_tensor("ones", [P, P], STAT, kind="ExternalInput")
    ident = nc.dram_tensor("ident", [P, P], STAT, kind="ExternalInput")
    y = nc.dram_tensor("y", [NB, C], F16, kind="ExternalOutput")
    io = (xT, wq, wk, wv, wo, kpT, vpn, cosq, sinq, cosk, sink, tri,
          rperm, ones, ident, y)
    with nc.allow_low_precision(reason="fp16 operands; fp32 accumulation"):
        with tile.TileContext(nc) as tc:
            _emit(tc, io)
    nc.compile()
    return nc


def host_inputs(x, Wq, Wkv, Wo, K_pool, V_pool, slot_map, past_len):
    x = np.asarray(x, dtype=np.float32)
    Wq = np.asarray(Wq, dtype=np.float32)
    Wkv = np.asarray(Wkv, dtype=np.float32)
    Wo = np.asarray(Wo, dtype=np.float32)
    K_pool = np.asarray(K_pool, dtype=np.float32)
    V_pool = np.asarray(V_pool, dtype=np.float32)
    slot_map = np.asarray(slot_map, dtype=np.int32)
    past = int(past_len)
    assert past == PAST, f"kernel hardcodes past_len={PAST}, got {past}"

    # [p, kc, tok]: per-partition contiguous x tiles
    xT = np.ascontiguousarray(
        x.reshape(NB, C).T.reshape(16, P, NB).transpose(1, 0, 2)
        .astype(np.float16))

    # rope tables; argument arithmetic mirrors the f32 ops of the reference
    idx = np.arange(D // 2, dtype=np.float32)
    inv = np.float32(1.0) / np.float32(10000.0) ** (idx / np.float32(D // 2))
    inv = inv.astype(np.float32)
    t = np.arange(past, past + T, dtype=np.float32)
    freqs = (t[:, None] * inv[None, :]).astype(np.float32)
    emb = np.concatenate([freqs, freqs], axis=1)
    cos = np.cos(emb).astype(np.float32)
    sin = np.sin(emb).astype(np.float32)
    qscale = np.float32(1.0) / np.sqrt(np.float32(D))
    cosqT = np.ascontiguousarray((cos * qscale).T.astype(np.float16))
    sinqT = np.ascontiguousarray((sin * qscale).T.astype(np.float16))
    coskT = np.ascontiguousarray(cos.T.astype(np.float16))
    sinkT = np.ascontiguousarray(sin.T.astype(np.float16))

    # shared [128,128] triangular mask for block-aligned causal diagonals
    s_i = np.arange(P)[:, None]
    u_i = np.arange(P)[None, :]
    tri = np.where(s_i <= u_i, 0.0, NEG).astype(np.float16)

    rperm = np.zeros((P, P), np.float32)
    for d in range(D // 2):
        rperm[d + D // 2, d] = -1.0       # rot(q)[d] = -q[d+64] for d < 64
        rperm[d, d + D // 2] = 1.0        # rot(q)[d] = q[d-64] for d >= 64
    rperm = rperm.astype(NP_BF16)
    ones = np.ones((P, P), NP_BF16)
    ident = np.eye(P, dtype=np.float32).astype(NP_BF16)

    # host-side past-KV gather (+ transpose for K): logical past order
    gs = np.asarray(slot_map[:, :past], dtype=np.int64)     # [B, 1024]
    in_maps = []
    for c in range(NCORES):
        Kg = K_pool[gs, c, :].astype(NP_BF16)               # [B, 1024, 128]
        Vg = V_pool[gs, c, :].astype(NP_BF16)
        # kT_past [d, b, j, s%128]  -> flat [128, B*8*128]
        kpT = np.ascontiguousarray(
            Kg.reshape(B, 8, P, D).transpose(3, 0, 1, 2).reshape(P, B * 8 * P))
        # vg [s%128, b, j, d] -> flat [128, B*8*128]
        vpn = np.ascontiguousarray(
            Vg.reshape(B, 8, P, D).transpose(2, 0, 1, 3).reshape(P, B * 8 * P))
        # weight tiles pre-arranged to [partition, kc*m] so device loads are
        # one contiguous run per partition
        wq_l = Wq[G * D * c:G * D * (c + 1), :].T.reshape(16, P, G * D)
        wq_l = wq_l.transpose(1, 0, 2).reshape(P, 16 * G * D)
        wk_l = Wkv[D * c:D * (c + 1), :].T.reshape(16, P, D)
        wk_l = wk_l.transpose(1, 0, 2).reshape(P, 16 * D)
        wv_l = Wkv[HKV * D + D * c:HKV * D + D * (c + 1), :].T.reshape(16, P, D)
        wv_l = wv_l.transpose(1, 0, 2).reshape(P, 16 * D)
        wo_l = Wo[:, G * D * c:G * D * (c + 1)].T.reshape(G, P, C)
        wo_l = wo_l.transpose(1, 0, 2).reshape(P, G * C)
        in_maps.append({
            "xT": xT,
            "wq": np.ascontiguousarray(wq_l.astype(NP_BF16)),
            "wk": np.ascontiguousarray(wk_l.astype(NP_BF16)),
            "wv": np.ascontiguousarray(wv_l.astype(NP_BF16)),
            "wo": np.ascontiguousarray(wo_l.astype(np.float16)),
            "kpT": kpT, "vpn": vpn,
            "cosq": cosqT, "sinq": sinqT, "cosk": coskT, "sink": sinkT,
            "tri": tri, "rperm": rperm, "ones": ones, "ident": ident,
        })
    return in_maps


_NC_CACHE = None


def kernel(**inputs):
    global _NC_CACHE
    in_maps = host_inputs(**inputs)
    if _NC_CACHE is None:
        _NC_CACHE = build_nc()
    res = run_bass_kernel_spmd(_NC_CACHE, in_maps, core_ids=list(range(NCORES)))
    y = res.results[0]["y"].astype(np.float32)
    for c in range(1, NCORES):
        y = y + res.results[c]["y"].astype(np.float32)
    return y.reshape(B, T, C)
